# revision 1
# baseline (speedup 1.0000x reference)
"""Trainium2 Bass kernel: CNN-feature SoftDTW few-shot classifier.

Computes, for Q=100 query sequences and S=25 support sequences (T=128 steps,
D=2048 features): pairwise squared-euclidean cost matrices, soft-DTW alignment
cost per (query, support) pair, then per-class mean distances -> logits.

Key numerical fact: with gamma=0.1 and cost magnitudes ~4096, the reference's
fp32 softmin is bitwise the hard min (exp((m-x)/gamma) underflows for every
non-minimal branch), so the DP is computed with min/add only. Each DP row is
one `tensor_tensor_scan(op0=min, op1=add)` instruction.

Sharding: data-parallel over queries, 13 per core (Q padded 100->104),
supports replicated. Per core:
  - PE: xy = (-2X)@Y^T in bf16 (16 K-tiles) + fp32r rank-2 update adding
    x2[i] + y2[s,j] exactly -> full cost matrix D in PSUM (fp32).
  - ACT: evacuate PSUM -> SBUF; DMA D to DRAM scratch per query.
  - DMA gather: re-layout D from [i, (s,j)] to [(q,s)-partition, i-window, j].
  - DVE: hard-DTW rows: min(up,diag) + scan, 128 rows x 3 pair-streams.
Host: bf16 packing/transposes, x2/y2 sums, final class-mean logits.
"""

import sys

for _p in ("/opt/trn_rl_repo",):
    if _p not in sys.path:
        sys.path.insert(0, _p)

import numpy as np
import ml_dtypes

# Problem shape (hardcoded: harness runs kernel.py standalone)
Q, S, T, DD = 100, 25, 128, 2048
NCORES = 8
QC = 13                 # queries per core; Q padded to 104
QPAD = QC * NCORES
NK = DD // 128          # 16 bf16 contraction tiles
SJ = S * T              # 3200 = flattened (support, j)
B = QC * S              # 325 pairs per core
# DP pair-tile streams aligned to query boundaries (offset, count):
# a stream's first row can run as soon as its LAST query's cost matrix is
# in DRAM, so query-aligned splits start streams as early as possible.
PT = [(0, 125), (125, 100), (225, 100)]   # q0-4 | q5-8 | q9-12
# 5/4/4 split: stream deps land at ~157/248/338us (query cadence ~23us), so
# the DVE picks up each stream right as its last query's D lands — the final
# stream starts at its data dependency instead of queueing behind stream 1.
W = 16                  # DP row-window per gather DMA
CH = 512                # matmul moving-chunk / PSUM bank width
# all chunks >=256 so the fp32r rank-2 matmul stays at 1 cycle/row
_CW = [512, 512, 512, 512, 512, 384, 256]
CHUNKS = [(sum(_CW[:i]), w) for i, w in enumerate(_CW)]
assert sum(_CW) == SJ
BIG = 1e10

_built = None          # cached compiled Bass program
_last_result = None    # last BassKernelResults (exec_time_ns when traced)
_predicted_ns = None   # Tile cost-model makespan of the per-core program


def _build():
    import concourse.bacc as bacc
    import concourse.mybir as mybir
    import concourse.tile as tile

    f32 = mybir.dt.float32
    f32r = mybir.dt.float32r
    bf16 = mybir.dt.bfloat16
    MIN = mybir.AluOpType.min
    ADD = mybir.AluOpType.add

    global _predicted_ns
    nc = bacc.Bacc("TRN2", debug=False)

    xt_d = nc.dram_tensor("xt", [QC, 128, NK * T], bf16, kind="ExternalInput")
    yt_d = nc.dram_tensor("yt", [128, NK * SJ], bf16, kind="ExternalInput")
    augl_d = nc.dram_tensor("augl", [QC, 2, T], f32r, kind="ExternalInput")
    augr_d = nc.dram_tensor("augr", [2, SJ], f32r, kind="ExternalInput")
    out_d = nc.dram_tensor("out_cd", [QC, S], f32, kind="ExternalOutput")
    # cost matrices staged pair-major: [q, s, i, j] -> window reads are
    # single 3-dim APs with 8KB-contiguous runs per pair
    dsc = nc.dram_tensor("dsc", [QC, S, T, T], f32)
    dsc_p = dsc[:].rearrange("q s i j -> (q s) i j")

    with tile.TileContext(nc) as tc:
        with (
            tc.tile_pool(name="const", bufs=1) as constp,
            tc.tile_pool(name="xq", bufs=2) as xqp,
            tc.tile_pool(name="augq", bufs=2) as augqp,
            tc.tile_pool(name="psum", bufs=8, space="PSUM") as psump,
            tc.tile_pool(name="dq", bufs=1) as dqp,
            tc.tile_pool(name="ga", bufs=2) as gap,      # pair-tiles 0 and 2
            tc.tile_pool(name="gb", bufs=2) as gbp,      # pair-tile 1
            tc.tile_pool(name="muda", bufs=2) as mudap,
            tc.tile_pool(name="mudb", bufs=2) as mudbp,
            tc.tile_pool(name="dp", bufs=1) as dpp,
        ):
            # q0's operands first on the ACT queue (ahead of the yt halves).
            xt0_sb = xqp.tile([128, NK * T], bf16, tag="xt")
            nc.scalar.dma_start(xt0_sb[:], xt_d[0])
            augl0_sb = augqp.tile([2, T], f32r, tag="augl")
            nc.scalar.dma_start(augl0_sb[:], augl_d[0])

            # Resident Y^T (bf16), per K-tile so q0 starts after ~800KB, and
            # alternated across the two physical HWDGE rings (SP + ACT FIFOs)
            # so the 13MB load streams at double the single-FIFO rate.
            yt_sb = constp.tile([128, NK * SJ], bf16)
            for k in range(NK):
                qeng = nc.sync if k % 2 == 0 else nc.scalar
                qeng.dma_start(yt_sb[:, k * SJ:(k + 1) * SJ],
                               yt_d[:, k * SJ:(k + 1) * SJ])
            augr_sb = constp.tile([2, SJ], f32r)
            nc.sync.dma_start(augr_sb[:], augr_d[:])

            # ---- Stage A: cost matrices, one query at a time ----
            for q in range(QC):
                if q == 0:
                    xt_sb, augl_sb = xt0_sb, augl0_sb
                else:
                    # scalar (ACT) HWDGE queue: out of the SP FIFO.
                    xt_sb = xqp.tile([128, NK * T], bf16, tag="xt")
                    nc.scalar.dma_start(xt_sb[:], xt_d[q])
                    augl_sb = augqp.tile([2, T], f32r, tag="augl")
                    nc.scalar.dma_start(augl_sb[:], augl_d[q])

                dq_sb = dqp.tile([128, SJ], f32, tag="dq")
                if q == 0:
                    # k-OUTER while the 16 yt K-tile loads stream in: every
                    # arriving K-tile feeds all 7 chunks (7 PSUM banks live),
                    # so q0's matrix completes with the prologue instead of
                    # 22us after it. Per-cell accumulation order is unchanged.
                    pss = []
                    for _ci in range(len(CHUNKS)):
                        ps_q0 = psump.tile([128, CH], f32, tag="ps")
                        pss.append(ps_q0)
                    for k in range(NK):
                        for ci, (c0, cw) in enumerate(CHUNKS):
                            nc.tensor.matmul(
                                pss[ci][:, :cw],
                                xt_sb[:, k * T:(k + 1) * T],
                                yt_sb[:, k * SJ + c0: k * SJ + c0 + cw],
                                start=(k == 0),
                                stop=False,
                            )
                    for ci, (c0, cw) in enumerate(CHUNKS):
                        nc.tensor.matmul(
                            pss[ci][:, :cw],
                            augl_sb[:, :],
                            augr_sb[:, c0:c0 + cw],
                            start=False,
                            stop=True,
                        )
                        nc.scalar.copy(dq_sb[:, c0:c0 + cw], pss[ci][:, :cw])
                        nc.sync.dma_start(
                            dsc[q, c0 // T:(c0 + cw) // T]
                            .rearrange("s i j -> i s j"),
                            dq_sb[:, c0:c0 + cw]
                            .rearrange("i (s j) -> i s j", j=T))
                else:
                    for c0, cw in CHUNKS:
                        ps = psump.tile([128, CH], f32, tag="ps")
                        for k in range(NK):
                            nc.tensor.matmul(
                                ps[:, :cw],
                                xt_sb[:, k * T:(k + 1) * T],
                                yt_sb[:, k * SJ + c0: k * SJ + c0 + cw],
                                start=(k == 0),
                                stop=False,
                            )
                        # rank-2 fp32 update: + ones*y2[s,j] + x2[i]*ones
                        nc.tensor.matmul(
                            ps[:, :cw],
                            augl_sb[:, :],
                            augr_sb[:, c0:c0 + cw],
                            start=False,
                            stop=True,
                        )
                        nc.scalar.copy(dq_sb[:, c0:c0 + cw], ps[:, :cw])
                        # per-chunk dsc write (chunk widths are whole
                        # s-blocks): the last piece lands ~3.5us after the
                        # last evac instead of a 5us whole-query DMA.
                        nc.sync.dma_start(
                            dsc[q, c0 // T:(c0 + cw) // T]
                            .rearrange("s i j -> i s j"),
                            dq_sb[:, c0:c0 + cw]
                            .rearrange("i (s j) -> i s j", j=T))

            # ---- Stage B: hard-DTW wavefront, 3 batched pair-tiles ----
            out_flat = out_d[:].rearrange("q s -> (q s)")
            for pt, (p0, np_) in enumerate(PT):
                eng = nc.vector  # Pool lacks 2-input TensorTensor on TRN2
                gpool = gbp if pt == 1 else gap
                mudp = mudbp if pt == 1 else mudap
                qa, qb = p0 // S, (p0 + np_ - 1) // S  # query range (aligned)

                r_a = dpp.tile([128, T + 4], f32, tag=f"ra{pt}")
                r_b = dpp.tile([128, T + 4], f32, tag=f"rb{pt}")
                # row 0: [0, BIG, BIG, ...]; r_b border col = BIG.
                # memsets on Pool: keeps them off the DVE critical chain.
                nc.gpsimd.memset(r_a[:np_, 1:T + 1], BIG)
                nc.gpsimd.memset(r_a[:np_, 0:1], 0.0)
                nc.gpsimd.memset(r_b[:np_, 0:1], BIG)

                g_tiles = {}
                for i in range(T):
                    if i % W == 0:
                        g_t = gpool.tile([128, W * T], f32, tag=f"g{pt % 2}")
                        g_tiles[i // W] = g_t
                        # One DMA per window (full SDMA-engine spread).
                        # Pool/SWDGE: idle sequencer, not paced by ACT/SP.
                        # Window 0 of the last stream splits off the final
                        # query so the earlier queries prefetch while q12's
                        # matrix is still being written.
                        if pt == len(PT) - 1 and i == 0:
                            cut = np_ - S
                            nc.gpsimd.dma_start(
                                g_t[:cut, :].rearrange("p (w j) -> p w j", j=T),
                                dsc_p[p0:p0 + cut, i:i + W, :],
                            )
                            nc.gpsimd.dma_start(
                                g_t[cut:np_, :].rearrange(
                                    "p (w j) -> p w j", j=T),
                                dsc_p[p0 + cut:p0 + np_, i:i + W, :],
                            )
                        else:
                            nc.gpsimd.dma_start(
                                g_t[:np_, :].rearrange("p (w j) -> p w j", j=T),
                                dsc_p[p0:p0 + np_, i:i + W, :],
                            )
                    g_t = g_tiles[i // W]
                    prev, cur = (r_a, r_b) if i % 2 == 0 else (r_b, r_a)
                    mud = mudp.tile([128, T], f32, tag=f"m{pt % 2}")
                    eng.tensor_tensor(
                        mud[:np_, :], prev[:np_, 1:T + 1], prev[:np_, 0:T], MIN)
                    eng.tensor_tensor_scan(
                        cur[:np_, 1:T + 1], mud[:np_, :],
                        g_t[:np_, (i % W) * T:(i % W + 1) * T],
                        BIG, MIN, ADD)
                    if i == 0:
                        # row-0 buffer becomes an interior row: border 0 -> BIG
                        eng.memset(prev[:np_, 0:1], BIG)

                final = r_b if T % 2 == 1 else r_a  # T=128 even -> last cur=r_a
                nc.sync.dma_start(out_flat[p0:p0 + np_], final[:np_, T:T + 1])

    ents = getattr(tc, "_perfetto_entries", None)
    if ents:
        _predicted_ns = int(max(e[2] for e in ents))
    nc.compile()
    return nc


def _pack_inputs(X, Yf):
    """Host-side packing into the exact SBUF layouts the kernel DMAs 1:1."""
    bf = ml_dtypes.bfloat16
    # xt[c]: [QC, 128(dk), NK*T] = bf16(-2*X)^T, K-tile-major free dim
    Xp = np.zeros((QPAD, T, DD), np.float32)
    Xp[:Q] = X
    xtq = np.ascontiguousarray(
        (-2.0 * Xp).astype(bf).transpose(0, 2, 1)        # [QPAD, DD, T]
        .reshape(QPAD, NK, 128, T).transpose(0, 2, 1, 3)  # [QPAD, 128, NK, T]
        .reshape(QPAD, 128, NK * T))
    # yt: [128(dk), NK*SJ] = bf16(Y)^T
    yt = np.ascontiguousarray(
        Yf.astype(bf).transpose(2, 0, 1)                 # [DD, S, T]
        .reshape(NK, 128, SJ).transpose(1, 0, 2)         # [128, NK, SJ]
        .reshape(128, NK * SJ))
    # exact fp32 norms
    x2 = np.einsum("qtd,qtd->qt", Xp, Xp, dtype=np.float32)  # [QPAD, T]
    y2 = np.einsum("std,std->st", Yf, Yf, dtype=np.float32)  # [S, T]
    augl = np.zeros((QPAD, 2, T), np.float32)
    augl[:, 0, :] = 1.0
    augl[:, 1, :] = x2
    augr = np.zeros((2, SJ), np.float32)
    augr[0] = y2.reshape(SJ)
    augr[1] = 1.0
    return xtq, yt, augl, augr


def kernel(support_features, support_labels, target_features, n_classes):
    global _built
    from concourse.bass_utils import run_bass_kernel_spmd

    X = np.asarray(target_features, dtype=np.float32)
    Yf = np.asarray(support_features, dtype=np.float32)
    labels = np.asarray(support_labels)
    ncls = int(np.asarray(n_classes))
    assert X.shape == (Q, T, DD) and Yf.shape == (S, T, DD), (
        f"kernel compiled for fixed shapes; got {X.shape}, {Yf.shape}")

    xtq, yt, augl, augr = _pack_inputs(X, Yf)

    if _built is None:
        _built = _build()
    nc = _built

    in_maps = [
        {
            "xt": np.ascontiguousarray(xtq[c * QC:(c + 1) * QC]),
            "yt": yt,
            "augl": np.ascontiguousarray(augl[c * QC:(c + 1) * QC]),
            "augr": augr,
        }
        for c in range(NCORES)
    ]
    res = run_bass_kernel_spmd(nc, in_maps, list(range(NCORES)))
    global _last_result
    _last_result = res
    cum = np.concatenate([res.results[c]["out_cd"] for c in range(NCORES)])[:Q]

    onehot = (labels[:, None] == np.arange(ncls)[None, :]).astype(np.float32)
    counts = np.maximum(onehot.sum(axis=0), 1.0).astype(np.float32)
    logits = -(cum.astype(np.float32) @ onehot) / counts
    return logits.astype(np.float32)



# revision 2
# speedup vs baseline: 2.2031x; 2.2031x over previous
"""Trainium2 Bass kernel: CNN-feature SoftDTW few-shot classifier (v2).

vs v1 (385us): fp8e4 DoubleRow matmuls (2x PE throughput; rel err ~1.6e-4
end-to-end, tolerance 2e-2), fp16 cost-matrix staging (half the DRAM
round-trip), and a bidirectional DTW wavefront (forward rows 0..63 +
backward rows 127..64 meeting at the middle) which halves the serial
DP tail after the last query's matmul.

Per core (13 queries x 25 supports = 325 pairs, 3 pair-streams):
  - PE: xy = (-2X)@Y^T fp8e4 DoubleRow (8 k-pair passes) + fp32r rank-2
    aug adding exact x2[i]+y2[s,j] -> D in PSUM fp32.
  - ACT: evacuate PSUM -> SBUF fp16; DMA D to DRAM scratch per chunk.
  - DMA gather (Pool/SWDGE): [pair, row-window, j] fp16 windows, fwd + bwd.
  - DVE: per stream two independent chains:
      fwd:  R[i,j] = D + min(R[i-1,j], R[i,j-1], R[i-1,j-1]), rows 0..63
      bwd:  B[i,j] = D + min(B[i+1,j], B[i,j+1], B[i+1,j+1]), rows 127..64
    each row = tensor_tensor(min) + tensor_tensor_scan(min,add); the bwd
    scan runs right-to-left via reversed APs. Combine:
      out = min_j F[63,j] + min(B[64,j], B[64,j+1]).
Host: fp8/fp32r packing, final class-mean logits.
"""

import sys

for _p in ("/opt/trn_rl_repo",):
    if _p not in sys.path:
        sys.path.insert(0, _p)

import numpy as np
import ml_dtypes

# Problem shape (hardcoded: harness runs kernel.py standalone)
Q, S, T, DD = 100, 25, 128, 2048
NCORES = 8
QC = 13                 # queries per core; Q padded to 104
QPAD = QC * NCORES
NK = DD // 128          # 16 k-tiles -> 8 fp8 DoubleRow pairs
SJ = S * T              # 3200 = flattened (support, j)
B = QC * S              # 325 pairs per core
# DP pair-tile streams aligned to query boundaries (offset, count)
PT = [(0, 75), (75, 125), (200, 125)]   # q0-2 | q3-7 | q8-12
W = 16                  # DP row-window per gather DMA
HT = T // 2             # 64 rows per direction
CH = 512                # matmul moving-chunk / PSUM bank width
_CW = [512, 512, 512, 512, 512, 384, 256]
CHUNKS = [(sum(_CW[:i]), w) for i, w in enumerate(_CW)]
assert sum(_CW) == SJ
# The whole DP runs at 1/16 scale (folded into the host packing, exact for
# fp32/fp8) so fp16 DP rows stay under 65504; BIG is the fp16-safe border.
BIG = 60000.0
SCALE = 16.0

_built = None
_last_result = None
_predicted_ns = None


def _build():
    import concourse.bacc as bacc
    import concourse.mybir as mybir
    import concourse.tile as tile

    f32 = mybir.dt.float32
    f32r = mybir.dt.float32r
    f16 = mybir.dt.float16
    fp8 = mybir.dt.float8e4
    MIN = mybir.AluOpType.min
    ADD = mybir.AluOpType.add
    DR = mybir.MatmulPerfMode.DoubleRow

    global _predicted_ns
    nc = bacc.Bacc("TRN2", debug=False)

    xt_d = nc.dram_tensor("xt", [QC, 128, NK * T], fp8, kind="ExternalInput")
    yt_d = nc.dram_tensor("yt", [128, NK * SJ], fp8, kind="ExternalInput")
    augl_d = nc.dram_tensor("augl", [QC, 2, T], f32r, kind="ExternalInput")
    augr_d = nc.dram_tensor("augr", [2, SJ], f32r, kind="ExternalInput")
    out_d = nc.dram_tensor("out_cd", [QC, S], f32, kind="ExternalOutput")
    dsc = nc.dram_tensor("dsc", [QC, S, T, T], f16)
    dsc_p = dsc[:].rearrange("q s i j -> (q s) i j")

    with tile.TileContext(nc) as tc:
        with (
            tc.tile_pool(name="const", bufs=1) as constp,
            tc.tile_pool(name="xq", bufs=2) as xqp,
            tc.tile_pool(name="augq", bufs=2) as augqp,
            tc.tile_pool(name="psum", bufs=8, space="PSUM") as psump,
            tc.tile_pool(name="dq", bufs=2) as dqp,
            tc.tile_pool(name="gf", bufs=2) as gfp,
            tc.tile_pool(name="gb", bufs=2) as gbp,
            tc.tile_pool(name="mud", bufs=2) as mudp,
            tc.tile_pool(name="dp", bufs=1) as dpp,
        ):
            # q0 operands first on the ACT queue
            xt0_sb = xqp.tile([128, NK * T], fp8, tag="xt")
            nc.scalar.dma_start(xt0_sb[:], xt_d[0])
            augl0_sb = augqp.tile([2, T], f32r, tag="augl")
            nc.scalar.dma_start(augl0_sb[:], augl_d[0])

            # Resident Y^T (fp8), per K-tile, alternated across the two
            # HWDGE rings so the 6.5MB load streams at double rate.
            yt_sb = constp.tile([128, NK * SJ], fp8)
            for k in range(NK):
                qeng = nc.sync if k % 2 == 0 else nc.scalar
                qeng.dma_start(yt_sb[:, k * SJ:(k + 1) * SJ],
                               yt_d[:, k * SJ:(k + 1) * SJ])
            augr_sb = constp.tile([2, SJ], f32r)
            nc.sync.dma_start(augr_sb[:], augr_d[:])

            yt_v = yt_sb[:].rearrange("p (k n) -> p k n", k=NK)

            # ---- Stage A: cost matrices, one query at a time ----
            for q in range(QC):
                if q == 0:
                    xt_sb, augl_sb = xt0_sb, augl0_sb
                else:
                    xt_sb = xqp.tile([128, NK * T], fp8, tag="xt")
                    nc.gpsimd.dma_start(xt_sb[:], xt_d[q])
                    augl_sb = augqp.tile([2, T], f32r, tag="augl")
                    nc.gpsimd.dma_start(augl_sb[:], augl_d[q])
                xt_v = xt_sb[:].rearrange("p (k t) -> p k t", k=NK)

                dq_sb = dqp.tile([128, SJ], f16, tag="dq")
                if q == 0:
                    # k-pair-OUTER while yt k-tiles stream in: each arriving
                    # pair feeds all 7 chunks (7 PSUM banks live).
                    pss = []
                    for _ci in range(len(CHUNKS)):
                        ps_q0 = psump.tile([128, CH], f32, tag="ps")
                        pss.append(ps_q0)
                    for kk in range(NK // 2):
                        for ci, (c0, cw) in enumerate(CHUNKS):
                            nc.tensor.matmul(
                                pss[ci][:, :cw],
                                xt_v[:, 2 * kk:2 * kk + 2, :],
                                yt_v[:, 2 * kk:2 * kk + 2, c0:c0 + cw],
                                start=(kk == 0), stop=False, perf_mode=DR)
                    for ci, (c0, cw) in enumerate(CHUNKS):
                        nc.tensor.matmul(
                            pss[ci][:, :cw], augl_sb[:, :],
                            augr_sb[:, c0:c0 + cw], start=False, stop=True)
                        nc.scalar.copy(dq_sb[:, c0:c0 + cw], pss[ci][:, :cw])
                        nc.sync.dma_start(
                            dsc[q, c0 // T:(c0 + cw) // T]
                            .rearrange("s i j -> i s j"),
                            dq_sb[:, c0:c0 + cw]
                            .rearrange("i (s j) -> i s j", j=T))
                else:
                    for c0, cw in CHUNKS:
                        ps = psump.tile([128, CH], f32, tag="ps")
                        for kk in range(NK // 2):
                            nc.tensor.matmul(
                                ps[:, :cw],
                                xt_v[:, 2 * kk:2 * kk + 2, :],
                                yt_v[:, 2 * kk:2 * kk + 2, c0:c0 + cw],
                                start=(kk == 0), stop=False, perf_mode=DR)
                        nc.tensor.matmul(
                            ps[:, :cw], augl_sb[:, :],
                            augr_sb[:, c0:c0 + cw], start=False, stop=True)
                        nc.scalar.copy(dq_sb[:, c0:c0 + cw], ps[:, :cw])
                        nc.sync.dma_start(
                            dsc[q, c0 // T:(c0 + cw) // T]
                            .rearrange("s i j -> i s j"),
                            dq_sb[:, c0:c0 + cw]
                            .rearrange("i (s j) -> i s j", j=T))

            # ---- Stage B: bidirectional hard-DTW, 3 pair-streams ----
            # r tiles hold BOTH direction rows (fp16, 1/16-scaled):
            #   col 0        : fwd left border
            #   cols 1..T    : fwd row  F[i, j=0..T-1]
            #   cols T+1..2T : bwd row  B[i, j=0..T-1]
            #   col 2T+1     : bwd right border
            # so one 3-dim-AP tensor_tensor computes both directions' muds
            # (min is commutative: both are min of the two 1-shifted slices).
            out_flat = out_d[:].rearrange("q s -> (q s)")
            W2 = 2 * T + 2
            for pt, (p0, np_) in enumerate(PT):
                eng = nc.vector

                r_a = dpp.tile([128, W2], f16, tag=f"ra{pt}")
                r_b = dpp.tile([128, W2], f16, tag=f"rb{pt}")
                # row -1 / row 128 are all-BIG; corners are 0.
                nc.gpsimd.memset(r_a[:np_, 1:2 * T + 1], BIG)
                nc.gpsimd.memset(r_a[:np_, 0:1], 0.0)
                nc.gpsimd.memset(r_a[:np_, 2 * T + 1:2 * T + 2], 0.0)
                nc.gpsimd.memset(r_b[:np_, 0:1], BIG)
                nc.gpsimd.memset(r_b[:np_, 2 * T + 1:2 * T + 2], BIG)

                # gather windows: fwd w covers rows [16w,16w+16),
                # bwd w covers rows [112-16w, 128-16w)
                gf_tiles, gb_tiles = {}, {}
                for w in range(HT // W):
                    for half, pool, tiles, base_row in (
                        ("f", gfp, gf_tiles, W * w),
                        ("b", gbp, gb_tiles, T - W * (w + 1)),
                    ):
                        g_t = pool.tile([128, W * T], f16, tag=f"g{half}{pt % 2}")
                        tiles[w] = g_t
                        if pt == len(PT) - 1 and w == 0:
                            # split off the final query so earlier queries
                            # prefetch while q12's matrix is still landing
                            cut = np_ - S
                            nc.gpsimd.dma_start(
                                g_t[:cut, :].rearrange("p (w j) -> p w j", j=T),
                                dsc_p[p0:p0 + cut,
                                      base_row:base_row + W, :])
                            nc.gpsimd.dma_start(
                                g_t[cut:np_, :].rearrange("p (w j) -> p w j", j=T),
                                dsc_p[p0 + cut:p0 + np_,
                                      base_row:base_row + W, :])
                        else:
                            nc.gpsimd.dma_start(
                                g_t[:np_, :].rearrange("p (w j) -> p w j", j=T),
                                dsc_p[p0:p0 + np_,
                                      base_row:base_row + W, :])

                def fused_mud(prev_t):
                    # seg0: min(F[i-1,j], F[i-1,j-1]); seg1: min(B[i+1,j], B[i+1,j+1])
                    m = mudp.tile([128, 2 * T], f16, tag=f"m{pt % 2}")
                    pv = prev_t[:np_, :].rearrange("p (s c) -> p s c", s=2)
                    mv = m[:np_, :].rearrange("p (s c) -> p s c", s=2)
                    eng.tensor_tensor(
                        mv[:, :, 0:T], pv[:, :, 1:T + 1], pv[:, :, 0:T], MIN)
                    return m

                for t in range(HT):
                    prev, cur = (r_a, r_b) if t % 2 == 0 else (r_b, r_a)
                    mud = fused_mud(prev)
                    # fwd row i=t (left-to-right)
                    eng.tensor_tensor_scan(
                        cur[:np_, 1:T + 1], mud[:np_, 0:T],
                        gf_tiles[t // W][:np_, (t % W) * T:(t % W + 1) * T],
                        BIG, MIN, ADD)
                    # bwd row i=127-t (right-to-left via reversed APs)
                    slot = (W - 1) - (t % W)
                    gb_row = gb_tiles[t // W][:np_, slot * T:(slot + 1) * T]
                    eng.tensor_tensor_scan(
                        cur[:np_, 2 * T:T:-1],
                        mud[:np_, 2 * T - 1:T - 1:-1],
                        gb_row[:, T - 1::-1], BIG, MIN, ADD)
                    if t == 0:
                        # row-0 buffers become interior rows: corners -> BIG
                        eng.memset(prev[:np_, 0:1], BIG)
                        eng.memset(prev[:np_, 2 * T + 1:2 * T + 2], BIG)

                # HT=64 even: last t=63 odd -> cur=r_a holds F[63] and B[64]
                fin = r_a if HT % 2 == 0 else r_b
                mud64 = fused_mud(fin)  # seg1 = min(B[64,j], B[64,j+1])
                comb = mudp.tile([128, T], f32, tag=f"c{pt % 2}")
                eng.tensor_tensor(
                    comb[:np_, :], fin[:np_, 1:T + 1], mud64[:np_, T:2 * T],
                    ADD)
                cd = dpp.tile([128, 1], f32, tag=f"cd{pt}")
                eng.tensor_reduce(cd[:np_, :], comb[:np_, :],
                                  mybir.AxisListType.X, MIN)
                nc.sync.dma_start(out_flat[p0:p0 + np_], cd[:np_, 0:1])

    ents = getattr(tc, "_perfetto_entries", None)
    if ents:
        _predicted_ns = int(max(e[2] for e in ents))
    nc.compile()
    return nc


def _pack_inputs(X, Yf):
    """Host-side packing into the exact SBUF layouts the kernel DMAs 1:1."""
    e4 = ml_dtypes.float8_e4m3
    # Whole pipeline at 1/16 scale: x,y each carry 1/4 (exact in fp8),
    # norms carry 1/16 (exact in fp32). Host rescales logits by 16.
    # xt: [QPAD, 128(dk), NK*T] = fp8(-2X/4)^T, K-tile-major free dim
    Xp = np.zeros((QPAD, T, DD), np.float32)
    Xp[:Q] = X
    xtq = np.ascontiguousarray(
        (-0.5 * Xp).astype(e4).transpose(0, 2, 1)         # [QPAD, DD, T]
        .reshape(QPAD, NK, 128, T).transpose(0, 2, 1, 3)  # [QPAD, 128, NK, T]
        .reshape(QPAD, 128, NK * T))
    # yt: [128(dk), NK*SJ] = fp8(Y/4)^T
    yt = np.ascontiguousarray(
        (0.25 * Yf).astype(e4).transpose(2, 0, 1)         # [DD, S, T]
        .reshape(NK, 128, SJ).transpose(1, 0, 2)          # [128, NK, SJ]
        .reshape(128, NK * SJ))
    # exact fp32 norms (1/16-scaled)
    x2 = np.einsum("qtd,qtd->qt", Xp, Xp, dtype=np.float32) / SCALE
    y2 = np.einsum("std,std->st", Yf, Yf, dtype=np.float32) / SCALE
    augl = np.zeros((QPAD, 2, T), np.float32)
    augl[:, 0, :] = 1.0
    augl[:, 1, :] = x2
    augr = np.zeros((2, SJ), np.float32)
    augr[0] = y2.reshape(SJ)
    augr[1] = 1.0
    return xtq, yt, augl, augr


def kernel(support_features, support_labels, target_features, n_classes):
    global _built
    from concourse.bass_utils import run_bass_kernel_spmd

    X = np.asarray(target_features, dtype=np.float32)
    Yf = np.asarray(support_features, dtype=np.float32)
    labels = np.asarray(support_labels)
    ncls = int(np.asarray(n_classes))
    assert X.shape == (Q, T, DD) and Yf.shape == (S, T, DD), (
        f"kernel compiled for fixed shapes; got {X.shape}, {Yf.shape}")

    xtq, yt, augl, augr = _pack_inputs(X, Yf)

    if _built is None:
        _built = _build()
    nc = _built

    in_maps = [
        {
            "xt": np.ascontiguousarray(xtq[c * QC:(c + 1) * QC]),
            "yt": yt,
            "augl": np.ascontiguousarray(augl[c * QC:(c + 1) * QC]),
            "augr": augr,
        }
        for c in range(NCORES)
    ]
    res = run_bass_kernel_spmd(nc, in_maps, list(range(NCORES)))
    global _last_result
    _last_result = res
    cum = np.concatenate([res.results[c]["out_cd"] for c in range(NCORES)])[:Q]

    onehot = (labels[:, None] == np.arange(ncls)[None, :]).astype(np.float32)
    counts = np.maximum(onehot.sum(axis=0), 1.0).astype(np.float32)
    logits = -(cum.astype(np.float32) * SCALE @ onehot) / counts
    return logits.astype(np.float32)


# revision 3
# speedup vs baseline: 2.2977x; 1.0429x over previous
"""Trainium2 Bass kernel: CNN-feature SoftDTW few-shot classifier (v2).

vs v1 (385us): fp8e4 DoubleRow matmuls (2x PE throughput; rel err ~1.6e-4
end-to-end, tolerance 2e-2), fp16 cost-matrix staging (half the DRAM
round-trip), and a bidirectional DTW wavefront (forward rows 0..63 +
backward rows 127..64 meeting at the middle) which halves the serial
DP tail after the last query's matmul.

Per core (13 queries x 25 supports = 325 pairs, 3 pair-streams):
  - PE: xy = (-2X)@Y^T fp8e4 DoubleRow (8 k-pair passes) + fp32r rank-2
    aug adding exact x2[i]+y2[s,j] -> D in PSUM fp32.
  - ACT: evacuate PSUM -> SBUF fp16; DMA D to DRAM scratch per chunk.
  - DMA gather (Pool/SWDGE): [pair, row-window, j] fp16 windows, fwd + bwd.
  - DVE: per stream two independent chains:
      fwd:  R[i,j] = D + min(R[i-1,j], R[i,j-1], R[i-1,j-1]), rows 0..63
      bwd:  B[i,j] = D + min(B[i+1,j], B[i,j+1], B[i+1,j+1]), rows 127..64
    each row = tensor_tensor(min) + tensor_tensor_scan(min,add); the bwd
    scan runs right-to-left via reversed APs. Combine:
      out = min_j F[63,j] + min(B[64,j], B[64,j+1]).
Host: fp8/fp32r packing, final class-mean logits.
"""

import sys

for _p in ("/opt/trn_rl_repo",):
    if _p not in sys.path:
        sys.path.insert(0, _p)

import numpy as np
import ml_dtypes

# Problem shape (hardcoded: harness runs kernel.py standalone)
Q, S, T, DD = 100, 25, 128, 2048
NCORES = 8
QC = 13                 # queries per core; Q padded to 104
QPAD = QC * NCORES
NK = DD // 128          # 16 k-tiles -> 8 fp8 DoubleRow pairs
SJ = S * T              # 3200 = flattened (support, j)
B = QC * S              # 325 pairs per core
# DP pair-tile streams aligned to query boundaries (offset, count)
PT = [(0, 75), (75, 125), (200, 125)]   # q0-2 | q3-7 | q8-12
W = 16                  # DP row-window per gather DMA
HT = T // 2             # 64 rows per direction
CH = 512                # matmul moving-chunk / PSUM bank width
_CW = [512, 512, 512, 512, 512, 384, 256]
CHUNKS = [(sum(_CW[:i]), w) for i, w in enumerate(_CW)]
assert sum(_CW) == SJ
# The whole DP runs at 1/16 scale (folded into the host packing, exact for
# fp32/fp8) so fp16 DP rows stay under 65504; BIG is the fp16-safe border.
BIG = 60000.0
SCALE = 16.0

_built = None
_last_result = None
_predicted_ns = None


def _build():
    import concourse.bacc as bacc
    import concourse.mybir as mybir
    import concourse.tile as tile

    f32 = mybir.dt.float32
    f32r = mybir.dt.float32r
    f16 = mybir.dt.float16
    fp8 = mybir.dt.float8e4
    MIN = mybir.AluOpType.min
    ADD = mybir.AluOpType.add
    DR = mybir.MatmulPerfMode.DoubleRow

    global _predicted_ns
    nc = bacc.Bacc("TRN2", debug=False)

    xt_d = nc.dram_tensor("xt", [QC, 128, NK * T], fp8, kind="ExternalInput")
    yt_d = nc.dram_tensor("yt", [128, NK * SJ], fp8, kind="ExternalInput")
    augl_d = nc.dram_tensor("augl", [QC, 2, T], f32r, kind="ExternalInput")
    augr_d = nc.dram_tensor("augr", [2, SJ], f32r, kind="ExternalInput")
    out_d = nc.dram_tensor("out_cd", [QC, S], f32, kind="ExternalOutput")
    dsc = nc.dram_tensor("dsc", [QC, S, T, T], f16)
    dsc_p = dsc[:].rearrange("q s i j -> (q s) i j")

    with tile.TileContext(nc) as tc:
        with (
            tc.tile_pool(name="const", bufs=1) as constp,
            tc.tile_pool(name="xq", bufs=2) as xqp,
            tc.tile_pool(name="augq", bufs=2) as augqp,
            tc.tile_pool(name="psum", bufs=8, space="PSUM") as psump,
            tc.tile_pool(name="dq", bufs=2) as dqp,
            tc.tile_pool(name="gf", bufs=2) as gfp,
            tc.tile_pool(name="gb", bufs=2) as gbp,
            tc.tile_pool(name="mud", bufs=2) as mudp,
            tc.tile_pool(name="dp", bufs=1) as dpp,
        ):
            # q0 operands first on the ACT queue
            xt0_sb = xqp.tile([128, NK * T], fp8, tag="xt")
            nc.scalar.dma_start(xt0_sb[:], xt_d[0])
            augl0_sb = augqp.tile([2, T], f32r, tag="augl")
            nc.scalar.dma_start(augl0_sb[:], augl_d[0])

            # Resident Y^T (fp8), per K-tile, alternated across the two
            # HWDGE rings so the 6.5MB load streams at double rate.
            yt_sb = constp.tile([128, NK * SJ], fp8)
            for k in range(NK):
                qeng = nc.sync if k % 2 == 0 else nc.scalar
                qeng.dma_start(yt_sb[:, k * SJ:(k + 1) * SJ],
                               yt_d[:, k * SJ:(k + 1) * SJ])
            augr_sb = constp.tile([2, SJ], f32r)
            nc.sync.dma_start(augr_sb[:], augr_d[:])

            yt_v = yt_sb[:].rearrange("p (k n) -> p k n", k=NK)

            # ---- Stage A: cost matrices, one query at a time ----
            for q in range(QC):
                if q == 0:
                    xt_sb, augl_sb = xt0_sb, augl0_sb
                else:
                    xt_sb = xqp.tile([128, NK * T], fp8, tag="xt")
                    nc.gpsimd.dma_start(xt_sb[:], xt_d[q])
                    augl_sb = augqp.tile([2, T], f32r, tag="augl")
                    nc.gpsimd.dma_start(augl_sb[:], augl_d[q])
                xt_v = xt_sb[:].rearrange("p (k t) -> p k t", k=NK)

                dq_sb = dqp.tile([128, SJ], f16, tag="dq")
                if q == 0:
                    # k-pair-OUTER while yt k-tiles stream in: each arriving
                    # pair feeds all 7 chunks (7 PSUM banks live).
                    pss = []
                    for _ci in range(len(CHUNKS)):
                        ps_q0 = psump.tile([128, CH], f32, tag="ps")
                        pss.append(ps_q0)
                    for kk in range(NK // 2):
                        for ci, (c0, cw) in enumerate(CHUNKS):
                            nc.tensor.matmul(
                                pss[ci][:, :cw],
                                xt_v[:, 2 * kk:2 * kk + 2, :],
                                yt_v[:, 2 * kk:2 * kk + 2, c0:c0 + cw],
                                start=(kk == 0), stop=False, perf_mode=DR)
                    for ci, (c0, cw) in enumerate(CHUNKS):
                        nc.tensor.matmul(
                            pss[ci][:, :cw], augl_sb[:, :],
                            augr_sb[:, c0:c0 + cw], start=False, stop=True)
                        nc.scalar.copy(dq_sb[:, c0:c0 + cw], pss[ci][:, :cw])
                        nc.sync.dma_start(
                            dsc[q, c0 // T:(c0 + cw) // T]
                            .rearrange("s i j -> i s j"),
                            dq_sb[:, c0:c0 + cw]
                            .rearrange("i (s j) -> i s j", j=T))
                else:
                    for c0, cw in CHUNKS:
                        ps = psump.tile([128, CH], f32, tag="ps")
                        for kk in range(NK // 2):
                            nc.tensor.matmul(
                                ps[:, :cw],
                                xt_v[:, 2 * kk:2 * kk + 2, :],
                                yt_v[:, 2 * kk:2 * kk + 2, c0:c0 + cw],
                                start=(kk == 0), stop=False, perf_mode=DR)
                        nc.tensor.matmul(
                            ps[:, :cw], augl_sb[:, :],
                            augr_sb[:, c0:c0 + cw], start=False, stop=True)
                        nc.scalar.copy(dq_sb[:, c0:c0 + cw], ps[:, :cw])
                        nc.sync.dma_start(
                            dsc[q, c0 // T:(c0 + cw) // T]
                            .rearrange("s i j -> i s j"),
                            dq_sb[:, c0:c0 + cw]
                            .rearrange("i (s j) -> i s j", j=T))

            # ---- Stage B: bidirectional hard-DTW, 3 pair-streams ----
            # r tiles hold BOTH direction rows (fp16, 1/16-scaled):
            #   col 0        : fwd left border
            #   cols 1..T    : fwd row  F[i, j=0..T-1]
            #   cols T+1..2T : bwd row  B[i, j=0..T-1]
            #   col 2T+1     : bwd right border
            # so one 3-dim-AP tensor_tensor computes both directions' muds
            # (min is commutative: both are min of the two 1-shifted slices).
            out_flat = out_d[:].rearrange("q s -> (q s)")
            W2 = 2 * T + 2
            st = []
            for pt, (p0, np_) in enumerate(PT):
                eng = nc.vector

                r_a = dpp.tile([128, W2], f16, tag=f"ra{pt}")
                r_b = dpp.tile([128, W2], f16, tag=f"rb{pt}")
                # row -1 / row 128 are all-BIG; corners are 0.
                nc.gpsimd.memset(r_a[:np_, 1:2 * T + 1], BIG)
                nc.gpsimd.memset(r_a[:np_, 0:1], 0.0)
                nc.gpsimd.memset(r_a[:np_, 2 * T + 1:2 * T + 2], 0.0)
                nc.gpsimd.memset(r_b[:np_, 0:1], BIG)
                nc.gpsimd.memset(r_b[:np_, 2 * T + 1:2 * T + 2], BIG)

                # gather windows: fwd w covers rows [16w,16w+16),
                # bwd w covers rows [112-16w, 128-16w)
                gf_tiles, gb_tiles = {}, {}
                for w in range(HT // W):
                    for half, pool, tiles, base_row in (
                        ("f", gfp, gf_tiles, W * w),
                        ("b", gbp, gb_tiles, T - W * (w + 1)),
                    ):
                        g_t = pool.tile([128, W * T], f16, tag=f"g{half}{pt % 2}")
                        tiles[w] = g_t
                        if pt == len(PT) - 1 and w == 0:
                            # split off the final query so earlier queries
                            # prefetch while q12's matrix is still landing
                            cut = np_ - S
                            nc.gpsimd.dma_start(
                                g_t[:cut, :].rearrange("p (w j) -> p w j", j=T),
                                dsc_p[p0:p0 + cut,
                                      base_row:base_row + W, :])
                            nc.gpsimd.dma_start(
                                g_t[cut:np_, :].rearrange("p (w j) -> p w j", j=T),
                                dsc_p[p0 + cut:p0 + np_,
                                      base_row:base_row + W, :])
                        else:
                            nc.gpsimd.dma_start(
                                g_t[:np_, :].rearrange("p (w j) -> p w j", j=T),
                                dsc_p[p0:p0 + np_,
                                      base_row:base_row + W, :])

                st.append((p0, np_, r_a, r_b, gf_tiles, gb_tiles))

            eng = nc.vector

            def fused_mud(pt, np_, prev_t):
                # seg0: min(F[i-1,j], F[i-1,j-1]); seg1: min(B[i+1,j], B[i+1,j+1])
                m = mudp.tile([128, 2 * T], f16, tag=f"m{pt}")
                pv = prev_t[:np_, :].rearrange("p (s c) -> p s c", s=2)
                mv = m[:np_, :].rearrange("p (s c) -> p s c", s=2)
                eng.tensor_tensor(
                    mv[:, :, 0:T], pv[:, :, 1:T + 1], pv[:, :, 0:T], MIN)
                return m

            # round-robin issue across streams: the DVE runs its scheduled
            # order in-order, so interleaving lets ready streams fill the
            # latency gaps of streams that are serial-chain-bound.
            for t in range(HT):
                for pt, (p0, np_, r_a, r_b, gf_tiles, gb_tiles) in enumerate(st):
                    prev, cur = (r_a, r_b) if t % 2 == 0 else (r_b, r_a)
                    mud = fused_mud(pt, np_, prev)
                    # fwd row i=t (left-to-right)
                    eng.tensor_tensor_scan(
                        cur[:np_, 1:T + 1], mud[:np_, 0:T],
                        gf_tiles[t // W][:np_, (t % W) * T:(t % W + 1) * T],
                        BIG, MIN, ADD)
                    # bwd row i=127-t (right-to-left via reversed APs)
                    slot = (W - 1) - (t % W)
                    gb_row = gb_tiles[t // W][:np_, slot * T:(slot + 1) * T]
                    eng.tensor_tensor_scan(
                        cur[:np_, 2 * T:T:-1],
                        mud[:np_, 2 * T - 1:T - 1:-1],
                        gb_row[:, T - 1::-1], BIG, MIN, ADD)
                    if t == 0:
                        # row-0 buffers become interior rows: corners -> BIG
                        eng.memset(prev[:np_, 0:1], BIG)
                        eng.memset(prev[:np_, 2 * T + 1:2 * T + 2], BIG)

            for pt, (p0, np_, r_a, r_b, gf_tiles, gb_tiles) in enumerate(st):
                # HT=64 even: last t=63 odd -> cur=r_a holds F[63] and B[64]
                fin = r_a if HT % 2 == 0 else r_b
                mud64 = fused_mud(pt, np_, fin)  # seg1 = min(B[64,*])
                comb = mudp.tile([128, T], f32, tag=f"c{pt % 2}")
                eng.tensor_tensor(
                    comb[:np_, :], fin[:np_, 1:T + 1], mud64[:np_, T:2 * T],
                    ADD)
                cd = dpp.tile([128, 1], f32, tag=f"cd{pt}")
                eng.tensor_reduce(cd[:np_, :], comb[:np_, :],
                                  mybir.AxisListType.X, MIN)
                nc.sync.dma_start(out_flat[p0:p0 + np_], cd[:np_, 0:1])

    ents = getattr(tc, "_perfetto_entries", None)
    if ents:
        _predicted_ns = int(max(e[2] for e in ents))
    nc.compile()
    return nc


def _pack_inputs(X, Yf):
    """Host-side packing into the exact SBUF layouts the kernel DMAs 1:1."""
    e4 = ml_dtypes.float8_e4m3
    # Whole pipeline at 1/16 scale: x,y each carry 1/4 (exact in fp8),
    # norms carry 1/16 (exact in fp32). Host rescales logits by 16.
    # xt: [QPAD, 128(dk), NK*T] = fp8(-2X/4)^T, K-tile-major free dim
    Xp = np.zeros((QPAD, T, DD), np.float32)
    Xp[:Q] = X
    xtq = np.ascontiguousarray(
        (-0.5 * Xp).astype(e4).transpose(0, 2, 1)         # [QPAD, DD, T]
        .reshape(QPAD, NK, 128, T).transpose(0, 2, 1, 3)  # [QPAD, 128, NK, T]
        .reshape(QPAD, 128, NK * T))
    # yt: [128(dk), NK*SJ] = fp8(Y/4)^T
    yt = np.ascontiguousarray(
        (0.25 * Yf).astype(e4).transpose(2, 0, 1)         # [DD, S, T]
        .reshape(NK, 128, SJ).transpose(1, 0, 2)          # [128, NK, SJ]
        .reshape(128, NK * SJ))
    # exact fp32 norms (1/16-scaled)
    x2 = np.einsum("qtd,qtd->qt", Xp, Xp, dtype=np.float32) / SCALE
    y2 = np.einsum("std,std->st", Yf, Yf, dtype=np.float32) / SCALE
    augl = np.zeros((QPAD, 2, T), np.float32)
    augl[:, 0, :] = 1.0
    augl[:, 1, :] = x2
    augr = np.zeros((2, SJ), np.float32)
    augr[0] = y2.reshape(SJ)
    augr[1] = 1.0
    return xtq, yt, augl, augr


def kernel(support_features, support_labels, target_features, n_classes):
    global _built
    from concourse.bass_utils import run_bass_kernel_spmd

    X = np.asarray(target_features, dtype=np.float32)
    Yf = np.asarray(support_features, dtype=np.float32)
    labels = np.asarray(support_labels)
    ncls = int(np.asarray(n_classes))
    assert X.shape == (Q, T, DD) and Yf.shape == (S, T, DD), (
        f"kernel compiled for fixed shapes; got {X.shape}, {Yf.shape}")

    xtq, yt, augl, augr = _pack_inputs(X, Yf)

    if _built is None:
        _built = _build()
    nc = _built

    in_maps = [
        {
            "xt": np.ascontiguousarray(xtq[c * QC:(c + 1) * QC]),
            "yt": yt,
            "augl": np.ascontiguousarray(augl[c * QC:(c + 1) * QC]),
            "augr": augr,
        }
        for c in range(NCORES)
    ]
    res = run_bass_kernel_spmd(nc, in_maps, list(range(NCORES)))
    global _last_result
    _last_result = res
    cum = np.concatenate([res.results[c]["out_cd"] for c in range(NCORES)])[:Q]

    onehot = (labels[:, None] == np.arange(ncls)[None, :]).astype(np.float32)
    counts = np.maximum(onehot.sum(axis=0), 1.0).astype(np.float32)
    logits = -(cum.astype(np.float32) * SCALE @ onehot) / counts
    return logits.astype(np.float32)


# revision 4
# speedup vs baseline: 2.3182x; 1.0089x over previous
"""Trainium2 Bass kernel: CNN-feature SoftDTW few-shot classifier (v2).

vs v1 (385us): fp8e4 DoubleRow matmuls (2x PE throughput; rel err ~1.6e-4
end-to-end, tolerance 2e-2), fp16 cost-matrix staging (half the DRAM
round-trip), and a bidirectional DTW wavefront (forward rows 0..63 +
backward rows 127..64 meeting at the middle) which halves the serial
DP tail after the last query's matmul.

Per core (13 queries x 25 supports = 325 pairs, 3 pair-streams):
  - PE: xy = (-2X)@Y^T fp8e4 DoubleRow (8 k-pair passes) + fp32r rank-2
    aug adding exact x2[i]+y2[s,j] -> D in PSUM fp32.
  - ACT: evacuate PSUM -> SBUF fp16; DMA D to DRAM scratch per chunk.
  - DMA gather (Pool/SWDGE): [pair, row-window, j] fp16 windows, fwd + bwd.
  - DVE: per stream two independent chains:
      fwd:  R[i,j] = D + min(R[i-1,j], R[i,j-1], R[i-1,j-1]), rows 0..63
      bwd:  B[i,j] = D + min(B[i+1,j], B[i,j+1], B[i+1,j+1]), rows 127..64
    each row = tensor_tensor(min) + tensor_tensor_scan(min,add); the bwd
    scan runs right-to-left via reversed APs. Combine:
      out = min_j F[63,j] + min(B[64,j], B[64,j+1]).
Host: fp8/fp32r packing, final class-mean logits.
"""

import sys

for _p in ("/opt/trn_rl_repo",):
    if _p not in sys.path:
        sys.path.insert(0, _p)

import numpy as np
import ml_dtypes

# Problem shape (hardcoded: harness runs kernel.py standalone)
Q, S, T, DD = 100, 25, 128, 2048
NCORES = 8
QC = 13                 # queries per core; Q padded to 104
QPAD = QC * NCORES
NK = DD // 128          # 16 k-tiles -> 8 fp8 DoubleRow pairs
SJ = S * T              # 3200 = flattened (support, j)
B = QC * S              # 325 pairs per core
# DP pair-tile streams aligned to query boundaries (offset, count)
PT = [(0, 75), (75, 125), (200, 125)]   # q0-2 | q3-7 | q8-12
W = 16                  # DP row-window per gather DMA
HT = T // 2             # 64 rows per direction
CH = 512                # matmul moving-chunk / PSUM bank width
_CW = [512, 512, 512, 512, 512, 384, 256]
CHUNKS = [(sum(_CW[:i]), w) for i, w in enumerate(_CW)]
assert sum(_CW) == SJ
# The whole DP runs at 1/16 scale (folded into the host packing, exact for
# fp32/fp8) so fp16 DP rows stay under 65504; BIG is the fp16-safe border.
BIG = 60000.0
SCALE = 16.0

_built = None
_last_result = None
_predicted_ns = None


def _build():
    import concourse.bacc as bacc
    import concourse.mybir as mybir
    import concourse.tile as tile

    f32 = mybir.dt.float32
    f32r = mybir.dt.float32r
    f16 = mybir.dt.float16
    fp8 = mybir.dt.float8e4
    MIN = mybir.AluOpType.min
    ADD = mybir.AluOpType.add
    DR = mybir.MatmulPerfMode.DoubleRow

    global _predicted_ns
    nc = bacc.Bacc("TRN2", debug=False)

    xt_d = nc.dram_tensor("xt", [QC, 128, NK * T], fp8, kind="ExternalInput")
    yt_d = nc.dram_tensor("yt", [128, NK * SJ], fp8, kind="ExternalInput")
    augl_d = nc.dram_tensor("augl", [QC, 2, T], f32r, kind="ExternalInput")
    augr_d = nc.dram_tensor("augr", [2, SJ], f32r, kind="ExternalInput")
    out_d = nc.dram_tensor("out_cd", [QC, S], f32, kind="ExternalOutput")
    dsc = nc.dram_tensor("dsc", [QC, S, T, T], f16)
    dsc_p = dsc[:].rearrange("q s i j -> (q s) i j")

    with tile.TileContext(nc) as tc:
        with (
            tc.tile_pool(name="const", bufs=1) as constp,
            tc.tile_pool(name="xq", bufs=2) as xqp,
            tc.tile_pool(name="augq", bufs=2) as augqp,
            tc.tile_pool(name="psum", bufs=8, space="PSUM") as psump,
            tc.tile_pool(name="dq", bufs=2) as dqp,
            tc.tile_pool(name="gf", bufs=3) as gfp,
            tc.tile_pool(name="gb", bufs=3) as gbp,
            tc.tile_pool(name="mud", bufs=2) as mudp,
            tc.tile_pool(name="dp", bufs=1) as dpp,
        ):
            # q0 operands first on the ACT queue
            xt0_sb = xqp.tile([128, NK * T], fp8, tag="xt")
            nc.scalar.dma_start(xt0_sb[:], xt_d[0])
            augl0_sb = augqp.tile([2, T], f32r, tag="augl")
            nc.scalar.dma_start(augl0_sb[:], augl_d[0])

            # Resident Y^T (fp8), per K-tile, alternated across the two
            # HWDGE rings so the 6.5MB load streams at double rate.
            yt_sb = constp.tile([128, NK * SJ], fp8)
            for k in range(NK):
                qeng = nc.sync if k % 2 == 0 else nc.scalar
                qeng.dma_start(yt_sb[:, k * SJ:(k + 1) * SJ],
                               yt_d[:, k * SJ:(k + 1) * SJ])
            augr_sb = constp.tile([2, SJ], f32r)
            nc.sync.dma_start(augr_sb[:], augr_d[:])

            yt_v = yt_sb[:].rearrange("p (k n) -> p k n", k=NK)

            # ---- Stage A: cost matrices, one query at a time ----
            for q in range(QC):
                if q == 0:
                    xt_sb, augl_sb = xt0_sb, augl0_sb
                else:
                    xt_sb = xqp.tile([128, NK * T], fp8, tag="xt")
                    nc.gpsimd.dma_start(xt_sb[:], xt_d[q])
                    augl_sb = augqp.tile([2, T], f32r, tag="augl")
                    nc.gpsimd.dma_start(augl_sb[:], augl_d[q])
                xt_v = xt_sb[:].rearrange("p (k t) -> p k t", k=NK)

                dq_sb = dqp.tile([128, SJ], f16, tag="dq")
                if q == 0:
                    # k-pair-OUTER while yt k-tiles stream in: each arriving
                    # pair feeds all 7 chunks (7 PSUM banks live).
                    pss = []
                    for _ci in range(len(CHUNKS)):
                        ps_q0 = psump.tile([128, CH], f32, tag="ps")
                        pss.append(ps_q0)
                    for kk in range(NK // 2):
                        for ci, (c0, cw) in enumerate(CHUNKS):
                            nc.tensor.matmul(
                                pss[ci][:, :cw],
                                xt_v[:, 2 * kk:2 * kk + 2, :],
                                yt_v[:, 2 * kk:2 * kk + 2, c0:c0 + cw],
                                start=(kk == 0), stop=False, perf_mode=DR)
                    for ci, (c0, cw) in enumerate(CHUNKS):
                        nc.tensor.matmul(
                            pss[ci][:, :cw], augl_sb[:, :],
                            augr_sb[:, c0:c0 + cw], start=False, stop=True)
                        nc.scalar.copy(dq_sb[:, c0:c0 + cw], pss[ci][:, :cw])
                        nc.sync.dma_start(
                            dsc[q, c0 // T:(c0 + cw) // T]
                            .rearrange("s i j -> i s j"),
                            dq_sb[:, c0:c0 + cw]
                            .rearrange("i (s j) -> i s j", j=T))
                else:
                    for c0, cw in CHUNKS:
                        ps = psump.tile([128, CH], f32, tag="ps")
                        for kk in range(NK // 2):
                            nc.tensor.matmul(
                                ps[:, :cw],
                                xt_v[:, 2 * kk:2 * kk + 2, :],
                                yt_v[:, 2 * kk:2 * kk + 2, c0:c0 + cw],
                                start=(kk == 0), stop=False, perf_mode=DR)
                        nc.tensor.matmul(
                            ps[:, :cw], augl_sb[:, :],
                            augr_sb[:, c0:c0 + cw], start=False, stop=True)
                        nc.scalar.copy(dq_sb[:, c0:c0 + cw], ps[:, :cw])
                        nc.sync.dma_start(
                            dsc[q, c0 // T:(c0 + cw) // T]
                            .rearrange("s i j -> i s j"),
                            dq_sb[:, c0:c0 + cw]
                            .rearrange("i (s j) -> i s j", j=T))

            # ---- Stage B: bidirectional hard-DTW, 3 pair-streams ----
            # r tiles hold BOTH direction rows (fp16, 1/16-scaled):
            #   col 0        : fwd left border
            #   cols 1..T    : fwd row  F[i, j=0..T-1]
            #   cols T+1..2T : bwd row  B[i, j=0..T-1]
            #   col 2T+1     : bwd right border
            # so one 3-dim-AP tensor_tensor computes both directions' muds
            # (min is commutative: both are min of the two 1-shifted slices).
            out_flat = out_d[:].rearrange("q s -> (q s)")
            W2 = 2 * T + 2
            st = []
            for pt, (p0, np_) in enumerate(PT):
                eng = nc.vector

                r_a = dpp.tile([128, W2], f16, tag=f"ra{pt}")
                r_b = dpp.tile([128, W2], f16, tag=f"rb{pt}")
                # row -1 / row 128 are all-BIG; corners are 0.
                nc.gpsimd.memset(r_a[:np_, 1:2 * T + 1], BIG)
                nc.gpsimd.memset(r_a[:np_, 0:1], 0.0)
                nc.gpsimd.memset(r_a[:np_, 2 * T + 1:2 * T + 2], 0.0)
                nc.gpsimd.memset(r_b[:np_, 0:1], BIG)
                nc.gpsimd.memset(r_b[:np_, 2 * T + 1:2 * T + 2], BIG)

                # gather windows: fwd w covers rows [16w,16w+16),
                # bwd w covers rows [112-16w, 128-16w)
                gf_tiles, gb_tiles = {}, {}
                for w in range(HT // W):
                    for half, pool, tiles, base_row in (
                        ("f", gfp, gf_tiles, W * w),
                        ("b", gbp, gb_tiles, T - W * (w + 1)),
                    ):
                        g_t = pool.tile([128, W * T], f16, tag=f"g{half}{pt % 2}")
                        tiles[w] = g_t
                        if pt == len(PT) - 1 and w == 0:
                            # split off the final query so earlier queries
                            # prefetch while q12's matrix is still landing
                            cut = np_ - S
                            nc.gpsimd.dma_start(
                                g_t[:cut, :].rearrange("p (w j) -> p w j", j=T),
                                dsc_p[p0:p0 + cut,
                                      base_row:base_row + W, :])
                            nc.gpsimd.dma_start(
                                g_t[cut:np_, :].rearrange("p (w j) -> p w j", j=T),
                                dsc_p[p0 + cut:p0 + np_,
                                      base_row:base_row + W, :])
                        else:
                            nc.gpsimd.dma_start(
                                g_t[:np_, :].rearrange("p (w j) -> p w j", j=T),
                                dsc_p[p0:p0 + np_,
                                      base_row:base_row + W, :])

                st.append((p0, np_, r_a, r_b, gf_tiles, gb_tiles))

            eng = nc.vector

            def fused_mud(pt, np_, prev_t):
                # seg0: min(F[i-1,j], F[i-1,j-1]); seg1: min(B[i+1,j], B[i+1,j+1])
                m = mudp.tile([128, 2 * T], f16, tag=f"m{pt}")
                pv = prev_t[:np_, :].rearrange("p (s c) -> p s c", s=2)
                mv = m[:np_, :].rearrange("p (s c) -> p s c", s=2)
                eng.tensor_tensor(
                    mv[:, :, 0:T], pv[:, :, 1:T + 1], pv[:, :, 0:T], MIN)
                return m

            # round-robin issue across streams: the DVE runs its scheduled
            # order in-order, so interleaving lets ready streams fill the
            # latency gaps of streams that are serial-chain-bound.
            for t in range(HT):
                for pt, (p0, np_, r_a, r_b, gf_tiles, gb_tiles) in enumerate(st):
                    prev, cur = (r_a, r_b) if t % 2 == 0 else (r_b, r_a)
                    if pt == 0:
                        # stream 0 runs mostly solo (other streams' data not
                        # staged yet): split the mud so the fwd/bwd chains
                        # decouple -- cadence becomes engine-bound (4 ops)
                        # instead of fused-chain latency-bound.
                        mud = mudp.tile([128, 2 * T], f16, tag=f"m{pt}")
                        eng.tensor_tensor(
                            mud[:np_, 0:T], prev[:np_, 1:T + 1],
                            prev[:np_, 0:T], MIN)
                        eng.tensor_tensor_scan(
                            cur[:np_, 1:T + 1], mud[:np_, 0:T],
                            gf_tiles[t // W][:np_, (t % W) * T:(t % W + 1) * T],
                            BIG, MIN, ADD)
                        eng.tensor_tensor(
                            mud[:np_, T:2 * T], prev[:np_, T + 1:2 * T + 1],
                            prev[:np_, T + 2:2 * T + 2], MIN)
                    else:
                        mud = fused_mud(pt, np_, prev)
                        # fwd row i=t (left-to-right)
                        eng.tensor_tensor_scan(
                            cur[:np_, 1:T + 1], mud[:np_, 0:T],
                            gf_tiles[t // W][:np_, (t % W) * T:(t % W + 1) * T],
                            BIG, MIN, ADD)
                    # bwd row i=127-t (right-to-left via reversed APs)
                    slot = (W - 1) - (t % W)
                    gb_row = gb_tiles[t // W][:np_, slot * T:(slot + 1) * T]
                    eng.tensor_tensor_scan(
                        cur[:np_, 2 * T:T:-1],
                        mud[:np_, 2 * T - 1:T - 1:-1],
                        gb_row[:, T - 1::-1], BIG, MIN, ADD)
                    if t == 0:
                        # row-0 buffers become interior rows: corners -> BIG
                        eng.memset(prev[:np_, 0:1], BIG)
                        eng.memset(prev[:np_, 2 * T + 1:2 * T + 2], BIG)

            for pt, (p0, np_, r_a, r_b, gf_tiles, gb_tiles) in enumerate(st):
                # HT=64 even: last t=63 odd -> cur=r_a holds F[63] and B[64]
                fin = r_a if HT % 2 == 0 else r_b
                mud64 = fused_mud(pt, np_, fin)  # seg1 = min(B[64,*])
                comb = mudp.tile([128, T], f32, tag=f"c{pt % 2}")
                eng.tensor_tensor(
                    comb[:np_, :], fin[:np_, 1:T + 1], mud64[:np_, T:2 * T],
                    ADD)
                cd = dpp.tile([128, 1], f32, tag=f"cd{pt}")
                eng.tensor_reduce(cd[:np_, :], comb[:np_, :],
                                  mybir.AxisListType.X, MIN)
                nc.sync.dma_start(out_flat[p0:p0 + np_], cd[:np_, 0:1])

    ents = getattr(tc, "_perfetto_entries", None)
    if ents:
        _predicted_ns = int(max(e[2] for e in ents))
    nc.compile()
    return nc


def _pack_inputs(X, Yf):
    """Host-side packing into the exact SBUF layouts the kernel DMAs 1:1."""
    e4 = ml_dtypes.float8_e4m3
    # Whole pipeline at 1/16 scale: x,y each carry 1/4 (exact in fp8),
    # norms carry 1/16 (exact in fp32). Host rescales logits by 16.
    # xt: [QPAD, 128(dk), NK*T] = fp8(-2X/4)^T, K-tile-major free dim
    Xp = np.zeros((QPAD, T, DD), np.float32)
    Xp[:Q] = X
    xtq = np.ascontiguousarray(
        (-0.5 * Xp).astype(e4).transpose(0, 2, 1)         # [QPAD, DD, T]
        .reshape(QPAD, NK, 128, T).transpose(0, 2, 1, 3)  # [QPAD, 128, NK, T]
        .reshape(QPAD, 128, NK * T))
    # yt: [128(dk), NK*SJ] = fp8(Y/4)^T
    yt = np.ascontiguousarray(
        (0.25 * Yf).astype(e4).transpose(2, 0, 1)         # [DD, S, T]
        .reshape(NK, 128, SJ).transpose(1, 0, 2)          # [128, NK, SJ]
        .reshape(128, NK * SJ))
    # exact fp32 norms (1/16-scaled)
    x2 = np.einsum("qtd,qtd->qt", Xp, Xp, dtype=np.float32) / SCALE
    y2 = np.einsum("std,std->st", Yf, Yf, dtype=np.float32) / SCALE
    augl = np.zeros((QPAD, 2, T), np.float32)
    augl[:, 0, :] = 1.0
    augl[:, 1, :] = x2
    augr = np.zeros((2, SJ), np.float32)
    augr[0] = y2.reshape(SJ)
    augr[1] = 1.0
    return xtq, yt, augl, augr


def kernel(support_features, support_labels, target_features, n_classes):
    global _built
    from concourse.bass_utils import run_bass_kernel_spmd

    X = np.asarray(target_features, dtype=np.float32)
    Yf = np.asarray(support_features, dtype=np.float32)
    labels = np.asarray(support_labels)
    ncls = int(np.asarray(n_classes))
    assert X.shape == (Q, T, DD) and Yf.shape == (S, T, DD), (
        f"kernel compiled for fixed shapes; got {X.shape}, {Yf.shape}")

    xtq, yt, augl, augr = _pack_inputs(X, Yf)

    if _built is None:
        _built = _build()
    nc = _built

    in_maps = [
        {
            "xt": np.ascontiguousarray(xtq[c * QC:(c + 1) * QC]),
            "yt": yt,
            "augl": np.ascontiguousarray(augl[c * QC:(c + 1) * QC]),
            "augr": augr,
        }
        for c in range(NCORES)
    ]
    res = run_bass_kernel_spmd(nc, in_maps, list(range(NCORES)))
    global _last_result
    _last_result = res
    cum = np.concatenate([res.results[c]["out_cd"] for c in range(NCORES)])[:Q]

    onehot = (labels[:, None] == np.arange(ncls)[None, :]).astype(np.float32)
    counts = np.maximum(onehot.sum(axis=0), 1.0).astype(np.float32)
    logits = -(cum.astype(np.float32) * SCALE @ onehot) / counts
    return logits.astype(np.float32)


# revision 5
# speedup vs baseline: 2.3620x; 1.0189x over previous
"""Trainium2 Bass kernel: CNN-feature SoftDTW few-shot classifier (v2).

vs v1 (385us -> 163us): fp8e4 DoubleRow matmuls (2x PE throughput; rel err ~1e-3
end-to-end, tolerance 2e-2), fp16 cost-matrix staging (half the DRAM
round-trip), and a bidirectional DTW wavefront (forward rows 0..63 +
backward rows 127..64 meeting at the middle) which halves the serial
DP tail after the last query's matmul.

Per core (13 queries x 25 supports = 325 pairs, 3 pair-streams):
  - PE: xy = (-2X)@Y^T fp8e4 DoubleRow (8 k-pair passes) + fp32r rank-2
    aug adding exact x2[i]+y2[s,j] -> D in PSUM fp32.
  - ACT: evacuate PSUM -> SBUF fp16; DMA D to DRAM scratch per chunk.
  - DMA gather (Pool/SWDGE): [pair, row-window, j] fp16 windows, fwd + bwd.
  - DVE: per stream two independent chains:
      fwd:  R[i,j] = D + min(R[i-1,j], R[i,j-1], R[i-1,j-1]), rows 0..63
      bwd:  B[i,j] = D + min(B[i+1,j], B[i,j+1], B[i+1,j+1]), rows 127..64
    each row = tensor_tensor(min) + tensor_tensor_scan(min,add); the bwd
    scan runs right-to-left via reversed APs. Combine:
      out = min_j F[63,j] + min(B[64,j], B[64,j+1]).
Host: fp8/fp32r packing, final class-mean logits.
"""

import sys

for _p in ("/opt/trn_rl_repo",):
    if _p not in sys.path:
        sys.path.insert(0, _p)

import numpy as np
import ml_dtypes

# Problem shape (hardcoded: harness runs kernel.py standalone)
Q, S, T, DD = 100, 25, 128, 2048
NCORES = 8
QC = 13                 # queries per core; Q padded to 104
QPAD = QC * NCORES
NK = DD // 128          # 16 k-tiles -> 8 fp8 DoubleRow pairs
SJ = S * T              # 3200 = flattened (support, j)
B = QC * S              # 325 pairs per core
# DP pair-tile streams aligned to query boundaries (offset, count)
PT = [(0, 75), (75, 125), (200, 125)]   # q0-2 | q3-7 | q8-12
W = 16                  # DP row-window per gather DMA
HT = T // 2             # 64 rows per direction
CH = 512                # matmul moving-chunk / PSUM bank width
_CW = [512, 512, 512, 512, 512, 384, 256]
CHUNKS = [(sum(_CW[:i]), w) for i, w in enumerate(_CW)]
assert sum(_CW) == SJ
# The whole DP runs at 1/16 scale (folded into the host packing, exact for
# fp32/fp8) so fp16 DP rows stay under 65504; BIG is the fp16-safe border.
BIG = 60000.0
SCALE = 16.0

_built = None
_last_result = None
_predicted_ns = None


def _build():
    import concourse.bacc as bacc
    import concourse.mybir as mybir
    import concourse.tile as tile

    f32 = mybir.dt.float32
    f32r = mybir.dt.float32r
    f16 = mybir.dt.float16
    fp8 = mybir.dt.float8e4
    MIN = mybir.AluOpType.min
    ADD = mybir.AluOpType.add
    DR = mybir.MatmulPerfMode.DoubleRow

    global _predicted_ns
    nc = bacc.Bacc("TRN2", debug=False)

    xt_d = nc.dram_tensor("xt", [QC, 128, NK * T], fp8, kind="ExternalInput")
    yt_d = nc.dram_tensor("yt", [128, NK * SJ], fp8, kind="ExternalInput")
    augl_d = nc.dram_tensor("augl", [QC, 2, T], f32r, kind="ExternalInput")
    augr_d = nc.dram_tensor("augr", [2, SJ], f32r, kind="ExternalInput")
    out_d = nc.dram_tensor("out_cd", [QC, S], f32, kind="ExternalOutput")
    dsc = nc.dram_tensor("dsc", [QC, S, T, T], f16)
    dsc_p = dsc[:].rearrange("q s i j -> (q s) i j")

    with tile.TileContext(nc) as tc:
        with (
            tc.tile_pool(name="const", bufs=1) as constp,
            tc.tile_pool(name="xq", bufs=2) as xqp,
            tc.tile_pool(name="augq", bufs=2) as augqp,
            tc.tile_pool(name="psum", bufs=8, space="PSUM") as psump,
            tc.tile_pool(name="dq", bufs=2) as dqp,
            tc.tile_pool(name="gf", bufs=3) as gfp,
            tc.tile_pool(name="gb", bufs=3) as gbp,
            tc.tile_pool(name="mud", bufs=2) as mudp,
            tc.tile_pool(name="dp", bufs=1) as dpp,
        ):
            # q0 operands first on the ACT queue
            xt0_sb = xqp.tile([128, NK * T], fp8, tag="xt")
            nc.scalar.dma_start(xt0_sb[:], xt_d[0])
            augl0_sb = augqp.tile([2, T], f32r, tag="augl")
            nc.scalar.dma_start(augl0_sb[:], augl_d[0])

            # Resident Y^T (fp8), per K-tile, alternated across the two
            # HWDGE rings so the 6.5MB load streams at double rate.
            yt_sb = constp.tile([128, NK * SJ], fp8)
            for k in range(NK):
                qeng = nc.sync if k % 2 == 0 else nc.scalar
                qeng.dma_start(yt_sb[:, k * SJ:(k + 1) * SJ],
                               yt_d[:, k * SJ:(k + 1) * SJ])
            augr_sb = constp.tile([2, SJ], f32r)
            nc.sync.dma_start(augr_sb[:], augr_d[:])

            yt_v = yt_sb[:].rearrange("p (k n) -> p k n", k=NK)

            # ---- Stage A: cost matrices, one query at a time ----
            for q in range(QC):
                if q == 0:
                    xt_sb, augl_sb = xt0_sb, augl0_sb
                else:
                    xt_sb = xqp.tile([128, NK * T], fp8, tag="xt")
                    nc.gpsimd.dma_start(xt_sb[:], xt_d[q])
                    augl_sb = augqp.tile([2, T], f32r, tag="augl")
                    nc.gpsimd.dma_start(augl_sb[:], augl_d[q])
                xt_v = xt_sb[:].rearrange("p (k t) -> p k t", k=NK)

                dq_sb = dqp.tile([128, SJ], f16, tag="dq")
                if q == 0:
                    # k-pair-OUTER while yt k-tiles stream in: each arriving
                    # pair feeds all 7 chunks (7 PSUM banks live).
                    pss = []
                    for _ci in range(len(CHUNKS)):
                        ps_q0 = psump.tile([128, CH], f32, tag="ps")
                        pss.append(ps_q0)
                    for kk in range(NK // 2):
                        for ci, (c0, cw) in enumerate(CHUNKS):
                            nc.tensor.matmul(
                                pss[ci][:, :cw],
                                xt_v[:, 2 * kk:2 * kk + 2, :],
                                yt_v[:, 2 * kk:2 * kk + 2, c0:c0 + cw],
                                start=(kk == 0), stop=False, perf_mode=DR)
                    for ci, (c0, cw) in enumerate(CHUNKS):
                        nc.tensor.matmul(
                            pss[ci][:, :cw], augl_sb[:, :],
                            augr_sb[:, c0:c0 + cw], start=False, stop=True)
                        nc.scalar.copy(dq_sb[:, c0:c0 + cw], pss[ci][:, :cw])
                        nc.sync.dma_start(
                            dsc[q, c0 // T:(c0 + cw) // T]
                            .rearrange("s i j -> i s j"),
                            dq_sb[:, c0:c0 + cw]
                            .rearrange("i (s j) -> i s j", j=T))
                else:
                    for c0, cw in CHUNKS:
                        ps = psump.tile([128, CH], f32, tag="ps")
                        for kk in range(NK // 2):
                            nc.tensor.matmul(
                                ps[:, :cw],
                                xt_v[:, 2 * kk:2 * kk + 2, :],
                                yt_v[:, 2 * kk:2 * kk + 2, c0:c0 + cw],
                                start=(kk == 0), stop=False, perf_mode=DR)
                        nc.tensor.matmul(
                            ps[:, :cw], augl_sb[:, :],
                            augr_sb[:, c0:c0 + cw], start=False, stop=True)
                        nc.scalar.copy(dq_sb[:, c0:c0 + cw], ps[:, :cw])
                        nc.sync.dma_start(
                            dsc[q, c0 // T:(c0 + cw) // T]
                            .rearrange("s i j -> i s j"),
                            dq_sb[:, c0:c0 + cw]
                            .rearrange("i (s j) -> i s j", j=T))

            # ---- Stage B: bidirectional hard-DTW, 3 pair-streams ----
            # r tiles hold BOTH direction rows (fp16, 1/16-scaled):
            #   col 0        : fwd left border
            #   cols 1..T    : fwd row  F[i, j=0..T-1]
            #   cols T+1..2T : bwd row  B[i, j=0..T-1]
            #   col 2T+1     : bwd right border
            # so one 3-dim-AP tensor_tensor computes both directions' muds
            # (min is commutative: both are min of the two 1-shifted slices).
            out_flat = out_d[:].rearrange("q s -> (q s)")
            W2 = 2 * T + 2
            st = []
            for pt, (p0, np_) in enumerate(PT):
                eng = nc.vector

                r_a = dpp.tile([128, W2], f16, tag=f"ra{pt}")
                r_b = dpp.tile([128, W2], f16, tag=f"rb{pt}")
                # row -1 / row 128 are all-BIG; corners are 0.
                nc.gpsimd.memset(r_a[:np_, 1:2 * T + 1], BIG)
                nc.gpsimd.memset(r_a[:np_, 0:1], 0.0)
                nc.gpsimd.memset(r_a[:np_, 2 * T + 1:2 * T + 2], 0.0)
                nc.gpsimd.memset(r_b[:np_, 0:1], BIG)
                nc.gpsimd.memset(r_b[:np_, 2 * T + 1:2 * T + 2], BIG)

                # gather windows: fwd w covers rows [16w,16w+16),
                # bwd w covers rows [112-16w, 128-16w)
                gf_tiles, gb_tiles = {}, {}
                for w in range(HT // W):
                    for half, pool, tiles, base_row in (
                        ("f", gfp, gf_tiles, W * w),
                        ("b", gbp, gb_tiles, T - W * (w + 1)),
                    ):
                        g_t = pool.tile([128, W * T], f16, tag=f"g{half}{pt % 2}")
                        tiles[w] = g_t
                        if pt == len(PT) - 1 and w == 0:
                            # split off the final query so earlier queries
                            # prefetch while q12's matrix is still landing
                            cut = np_ - S
                            nc.gpsimd.dma_start(
                                g_t[:cut, :].rearrange("p (w j) -> p w j", j=T),
                                dsc_p[p0:p0 + cut,
                                      base_row:base_row + W, :])
                            nc.gpsimd.dma_start(
                                g_t[cut:np_, :].rearrange("p (w j) -> p w j", j=T),
                                dsc_p[p0 + cut:p0 + np_,
                                      base_row:base_row + W, :])
                        else:
                            nc.gpsimd.dma_start(
                                g_t[:np_, :].rearrange("p (w j) -> p w j", j=T),
                                dsc_p[p0:p0 + np_,
                                      base_row:base_row + W, :])

                st.append((p0, np_, r_a, r_b, gf_tiles, gb_tiles))

            eng = nc.vector

            def fused_mud(pt, np_, prev_t):
                # seg0: min(F[i-1,j], F[i-1,j-1]); seg1: min(B[i+1,j], B[i+1,j+1])
                m = mudp.tile([128, 2 * T], f16, tag=f"m{pt}")
                pv = prev_t[:np_, :].rearrange("p (s c) -> p s c", s=2)
                mv = m[:np_, :].rearrange("p (s c) -> p s c", s=2)
                eng.tensor_tensor(
                    mv[:, :, 0:T], pv[:, :, 1:T + 1], pv[:, :, 0:T], MIN)
                return m

            # round-robin issue across streams: the DVE runs its scheduled
            # order in-order, so interleaving lets ready streams fill the
            # latency gaps of streams that are serial-chain-bound.
            for t in range(HT):
                for pt, (p0, np_, r_a, r_b, gf_tiles, gb_tiles) in enumerate(st):
                    prev, cur = (r_a, r_b) if t % 2 == 0 else (r_b, r_a)
                    if pt == 0 or (pt == 2 and t >= 42):
                        # stream 0 runs mostly solo (other streams' data not
                        # staged yet): split the mud so the fwd/bwd chains
                        # decouple -- cadence becomes engine-bound (4 ops)
                        # instead of fused-chain latency-bound.
                        mud = mudp.tile([128, 2 * T], f16, tag=f"m{pt}")
                        eng.tensor_tensor(
                            mud[:np_, 0:T], prev[:np_, 1:T + 1],
                            prev[:np_, 0:T], MIN)
                        eng.tensor_tensor_scan(
                            cur[:np_, 1:T + 1], mud[:np_, 0:T],
                            gf_tiles[t // W][:np_, (t % W) * T:(t % W + 1) * T],
                            BIG, MIN, ADD)
                        eng.tensor_tensor(
                            mud[:np_, T:2 * T], prev[:np_, T + 1:2 * T + 1],
                            prev[:np_, T + 2:2 * T + 2], MIN)
                    else:
                        mud = fused_mud(pt, np_, prev)
                        # fwd row i=t (left-to-right)
                        eng.tensor_tensor_scan(
                            cur[:np_, 1:T + 1], mud[:np_, 0:T],
                            gf_tiles[t // W][:np_, (t % W) * T:(t % W + 1) * T],
                            BIG, MIN, ADD)
                    # bwd row i=127-t (right-to-left via reversed APs)
                    slot = (W - 1) - (t % W)
                    gb_row = gb_tiles[t // W][:np_, slot * T:(slot + 1) * T]
                    eng.tensor_tensor_scan(
                        cur[:np_, 2 * T:T:-1],
                        mud[:np_, 2 * T - 1:T - 1:-1],
                        gb_row[:, T - 1::-1], BIG, MIN, ADD)
                    if t == 0:
                        # row-0 buffers become interior rows: corners -> BIG
                        eng.memset(prev[:np_, 0:1], BIG)
                        eng.memset(prev[:np_, 2 * T + 1:2 * T + 2], BIG)

            for pt, (p0, np_, r_a, r_b, gf_tiles, gb_tiles) in enumerate(st):
                # HT=64 even: last t=63 odd -> cur=r_a holds F[63] and B[64]
                fin = r_a if HT % 2 == 0 else r_b
                mud64 = fused_mud(pt, np_, fin)  # seg1 = min(B[64,*])
                comb = mudp.tile([128, T], f32, tag=f"c{pt % 2}")
                eng.tensor_tensor(
                    comb[:np_, :], fin[:np_, 1:T + 1], mud64[:np_, T:2 * T],
                    ADD)
                cd = dpp.tile([128, 1], f32, tag=f"cd{pt}")
                eng.tensor_reduce(cd[:np_, :], comb[:np_, :],
                                  mybir.AxisListType.X, MIN)
                nc.sync.dma_start(out_flat[p0:p0 + np_], cd[:np_, 0:1])

    ents = getattr(tc, "_perfetto_entries", None)
    if ents:
        _predicted_ns = int(max(e[2] for e in ents))
    nc.compile()
    return nc


def _pack_inputs(X, Yf):
    """Host-side packing into the exact SBUF layouts the kernel DMAs 1:1."""
    e4 = ml_dtypes.float8_e4m3
    # Whole pipeline at 1/16 scale: x,y each carry 1/4 (exact in fp8),
    # norms carry 1/16 (exact in fp32). Host rescales logits by 16.
    # xt: [QPAD, 128(dk), NK*T] = fp8(-2X/4)^T, K-tile-major free dim
    Xp = np.zeros((QPAD, T, DD), np.float32)
    Xp[:Q] = X
    xtq = np.ascontiguousarray(
        (-0.5 * Xp).astype(e4).transpose(0, 2, 1)         # [QPAD, DD, T]
        .reshape(QPAD, NK, 128, T).transpose(0, 2, 1, 3)  # [QPAD, 128, NK, T]
        .reshape(QPAD, 128, NK * T))
    # yt: [128(dk), NK*SJ] = fp8(Y/4)^T
    yt = np.ascontiguousarray(
        (0.25 * Yf).astype(e4).transpose(2, 0, 1)         # [DD, S, T]
        .reshape(NK, 128, SJ).transpose(1, 0, 2)          # [128, NK, SJ]
        .reshape(128, NK * SJ))
    # exact fp32 norms (1/16-scaled)
    x2 = np.einsum("qtd,qtd->qt", Xp, Xp, dtype=np.float32) / SCALE
    y2 = np.einsum("std,std->st", Yf, Yf, dtype=np.float32) / SCALE
    augl = np.zeros((QPAD, 2, T), np.float32)
    augl[:, 0, :] = 1.0
    augl[:, 1, :] = x2
    augr = np.zeros((2, SJ), np.float32)
    augr[0] = y2.reshape(SJ)
    augr[1] = 1.0
    return xtq, yt, augl, augr


def kernel(support_features, support_labels, target_features, n_classes):
    global _built
    from concourse.bass_utils import run_bass_kernel_spmd

    X = np.asarray(target_features, dtype=np.float32)
    Yf = np.asarray(support_features, dtype=np.float32)
    labels = np.asarray(support_labels)
    ncls = int(np.asarray(n_classes))
    assert X.shape == (Q, T, DD) and Yf.shape == (S, T, DD), (
        f"kernel compiled for fixed shapes; got {X.shape}, {Yf.shape}")

    xtq, yt, augl, augr = _pack_inputs(X, Yf)

    if _built is None:
        _built = _build()
    nc = _built

    in_maps = [
        {
            "xt": np.ascontiguousarray(xtq[c * QC:(c + 1) * QC]),
            "yt": yt,
            "augl": np.ascontiguousarray(augl[c * QC:(c + 1) * QC]),
            "augr": augr,
        }
        for c in range(NCORES)
    ]
    res = run_bass_kernel_spmd(nc, in_maps, list(range(NCORES)))
    global _last_result
    _last_result = res
    cum = np.concatenate([res.results[c]["out_cd"] for c in range(NCORES)])[:Q]

    onehot = (labels[:, None] == np.arange(ncls)[None, :]).astype(np.float32)
    counts = np.maximum(onehot.sum(axis=0), 1.0).astype(np.float32)
    logits = -(cum.astype(np.float32) * SCALE @ onehot) / counts
    return logits.astype(np.float32)


# revision 6
# speedup vs baseline: 2.3663x; 1.0018x over previous
"""Trainium2 Bass kernel: CNN-feature SoftDTW few-shot classifier (v2).

vs v1 (385us -> 163us): fp8e4 DoubleRow matmuls (2x PE throughput; rel err ~1e-3
end-to-end, tolerance 2e-2), fp16 cost-matrix staging (half the DRAM
round-trip), and a bidirectional DTW wavefront (forward rows 0..63 +
backward rows 127..64 meeting at the middle) which halves the serial
DP tail after the last query's matmul.

Per core (13 queries x 25 supports = 325 pairs, 3 pair-streams):
  - PE: xy = (-2X)@Y^T fp8e4 DoubleRow (8 k-pair passes) + fp32r rank-2
    aug adding exact x2[i]+y2[s,j] -> D in PSUM fp32.
  - ACT: evacuate PSUM -> SBUF fp16; DMA D to DRAM scratch per chunk.
  - DMA gather (Pool/SWDGE): [pair, row-window, j] fp16 windows, fwd + bwd.
  - DVE: per stream two independent chains:
      fwd:  R[i,j] = D + min(R[i-1,j], R[i,j-1], R[i-1,j-1]), rows 0..63
      bwd:  B[i,j] = D + min(B[i+1,j], B[i,j+1], B[i+1,j+1]), rows 127..64
    each row = tensor_tensor(min) + tensor_tensor_scan(min,add); the bwd
    scan runs right-to-left via reversed APs. Combine:
      out = min_j F[63,j] + min(B[64,j], B[64,j+1]).
Host: fp8/fp32r packing, final class-mean logits.
"""

import sys

for _p in ("/opt/trn_rl_repo",):
    if _p not in sys.path:
        sys.path.insert(0, _p)

import numpy as np
import ml_dtypes

# Problem shape (hardcoded: harness runs kernel.py standalone)
Q, S, T, DD = 100, 25, 128, 2048
NCORES = 8
QC = 13                 # queries per core; Q padded to 104
QPAD = QC * NCORES
NK = DD // 128          # 16 k-tiles -> 8 fp8 DoubleRow pairs
SJ = S * T              # 3200 = flattened (support, j)
B = QC * S              # 325 pairs per core
# DP pair-tile streams aligned to query boundaries (offset, count)
PT = [(0, 75), (75, 125), (200, 125)]   # q0-2 | q3-7 | q8-12
W = 16                  # DP row-window per gather DMA
HT = T // 2             # 64 rows per direction
CH = 512                # matmul moving-chunk / PSUM bank width
_CW = [512, 512, 512, 512, 512, 384, 256]
CHUNKS = [(sum(_CW[:i]), w) for i, w in enumerate(_CW)]
assert sum(_CW) == SJ
# The whole DP runs at 1/16 scale (folded into the host packing, exact for
# fp32/fp8) so fp16 DP rows stay under 65504; BIG is the fp16-safe border.
BIG = 60000.0
SCALE = 16.0

_built = None
_last_result = None
_predicted_ns = None


def _build():
    import concourse.bacc as bacc
    import concourse.mybir as mybir
    import concourse.tile as tile

    f32 = mybir.dt.float32
    f32r = mybir.dt.float32r
    f16 = mybir.dt.float16
    fp8 = mybir.dt.float8e4
    MIN = mybir.AluOpType.min
    ADD = mybir.AluOpType.add
    DR = mybir.MatmulPerfMode.DoubleRow

    global _predicted_ns
    nc = bacc.Bacc("TRN2", debug=False)

    xt_d = nc.dram_tensor("xt", [QC, 128, NK * T], fp8, kind="ExternalInput")
    yt_d = nc.dram_tensor("yt", [128, NK * SJ], fp8, kind="ExternalInput")
    augl_d = nc.dram_tensor("augl", [QC, 2, T], f32r, kind="ExternalInput")
    augr_d = nc.dram_tensor("augr", [2, SJ], f32r, kind="ExternalInput")
    out_d = nc.dram_tensor("out_cd", [QC, S], f32, kind="ExternalOutput")
    dsc = nc.dram_tensor("dsc", [QC, S, T, T], f16)
    dsc_p = dsc[:].rearrange("q s i j -> (q s) i j")

    with tile.TileContext(nc) as tc:
        with (
            tc.tile_pool(name="const", bufs=1) as constp,
            tc.tile_pool(name="xq", bufs=2) as xqp,
            tc.tile_pool(name="augq", bufs=2) as augqp,
            tc.tile_pool(name="psum", bufs=8, space="PSUM") as psump,
            tc.tile_pool(name="dq", bufs=2) as dqp,
            tc.tile_pool(name="gf", bufs=3) as gfp,
            tc.tile_pool(name="gb", bufs=3) as gbp,
            tc.tile_pool(name="mud", bufs=3) as mudp,
            tc.tile_pool(name="dp", bufs=1) as dpp,
        ):
            # q0 operands first on the ACT queue
            xt0_sb = xqp.tile([128, NK * T], fp8, tag="xt")
            nc.scalar.dma_start(xt0_sb[:], xt_d[0])
            augl0_sb = augqp.tile([2, T], f32r, tag="augl")
            nc.scalar.dma_start(augl0_sb[:], augl_d[0])

            # Resident Y^T (fp8), per K-tile, alternated across the two
            # HWDGE rings so the 6.5MB load streams at double rate.
            yt_sb = constp.tile([128, NK * SJ], fp8)
            for k in range(NK):
                qeng = nc.sync if k % 2 == 0 else nc.scalar
                qeng.dma_start(yt_sb[:, k * SJ:(k + 1) * SJ],
                               yt_d[:, k * SJ:(k + 1) * SJ])
            augr_sb = constp.tile([2, SJ], f32r)
            nc.sync.dma_start(augr_sb[:], augr_d[:])

            yt_v = yt_sb[:].rearrange("p (k n) -> p k n", k=NK)

            # ---- Stage A: cost matrices, one query at a time ----
            for q in range(QC):
                if q == 0:
                    xt_sb, augl_sb = xt0_sb, augl0_sb
                else:
                    xt_sb = xqp.tile([128, NK * T], fp8, tag="xt")
                    nc.gpsimd.dma_start(xt_sb[:], xt_d[q])
                    augl_sb = augqp.tile([2, T], f32r, tag="augl")
                    nc.gpsimd.dma_start(augl_sb[:], augl_d[q])
                xt_v = xt_sb[:].rearrange("p (k t) -> p k t", k=NK)

                dq_sb = dqp.tile([128, SJ], f16, tag="dq")
                if q == 0:
                    # k-pair-OUTER while yt k-tiles stream in: each arriving
                    # pair feeds all 7 chunks (7 PSUM banks live).
                    pss = []
                    for _ci in range(len(CHUNKS)):
                        ps_q0 = psump.tile([128, CH], f32, tag="ps")
                        pss.append(ps_q0)
                    for kk in range(NK // 2):
                        for ci, (c0, cw) in enumerate(CHUNKS):
                            nc.tensor.matmul(
                                pss[ci][:, :cw],
                                xt_v[:, 2 * kk:2 * kk + 2, :],
                                yt_v[:, 2 * kk:2 * kk + 2, c0:c0 + cw],
                                start=(kk == 0), stop=False, perf_mode=DR)
                    for ci, (c0, cw) in enumerate(CHUNKS):
                        nc.tensor.matmul(
                            pss[ci][:, :cw], augl_sb[:, :],
                            augr_sb[:, c0:c0 + cw], start=False, stop=True)
                        nc.scalar.copy(dq_sb[:, c0:c0 + cw], pss[ci][:, :cw])
                        nc.sync.dma_start(
                            dsc[q, c0 // T:(c0 + cw) // T]
                            .rearrange("s i j -> i s j"),
                            dq_sb[:, c0:c0 + cw]
                            .rearrange("i (s j) -> i s j", j=T))
                else:
                    for c0, cw in CHUNKS:
                        ps = psump.tile([128, CH], f32, tag="ps")
                        for kk in range(NK // 2):
                            nc.tensor.matmul(
                                ps[:, :cw],
                                xt_v[:, 2 * kk:2 * kk + 2, :],
                                yt_v[:, 2 * kk:2 * kk + 2, c0:c0 + cw],
                                start=(kk == 0), stop=False, perf_mode=DR)
                        nc.tensor.matmul(
                            ps[:, :cw], augl_sb[:, :],
                            augr_sb[:, c0:c0 + cw], start=False, stop=True)
                        nc.scalar.copy(dq_sb[:, c0:c0 + cw], ps[:, :cw])
                        nc.sync.dma_start(
                            dsc[q, c0 // T:(c0 + cw) // T]
                            .rearrange("s i j -> i s j"),
                            dq_sb[:, c0:c0 + cw]
                            .rearrange("i (s j) -> i s j", j=T))

            # ---- Stage B: bidirectional hard-DTW, 3 pair-streams ----
            # r tiles hold BOTH direction rows (fp16, 1/16-scaled):
            #   col 0        : fwd left border
            #   cols 1..T    : fwd row  F[i, j=0..T-1]
            #   cols T+1..2T : bwd row  B[i, j=0..T-1]
            #   col 2T+1     : bwd right border
            # so one 3-dim-AP tensor_tensor computes both directions' muds
            # (min is commutative: both are min of the two 1-shifted slices).
            out_flat = out_d[:].rearrange("q s -> (q s)")
            W2 = 2 * T + 2
            st = []
            for pt, (p0, np_) in enumerate(PT):
                eng = nc.vector

                r_a = dpp.tile([128, W2], f16, tag=f"ra{pt}")
                r_b = dpp.tile([128, W2], f16, tag=f"rb{pt}")
                # row -1 / row 128 are all-BIG; corners are 0.
                nc.gpsimd.memset(r_a[:np_, 1:2 * T + 1], BIG)
                nc.gpsimd.memset(r_a[:np_, 0:1], 0.0)
                nc.gpsimd.memset(r_a[:np_, 2 * T + 1:2 * T + 2], 0.0)
                nc.gpsimd.memset(r_b[:np_, 0:1], BIG)
                nc.gpsimd.memset(r_b[:np_, 2 * T + 1:2 * T + 2], BIG)

                # gather windows: fwd w covers rows [16w,16w+16),
                # bwd w covers rows [112-16w, 128-16w)
                gf_tiles, gb_tiles = {}, {}
                for w in range(HT // W):
                    for half, pool, tiles, base_row in (
                        ("f", gfp, gf_tiles, W * w),
                        ("b", gbp, gb_tiles, T - W * (w + 1)),
                    ):
                        g_t = pool.tile([128, W * T], f16, tag=f"g{half}{pt % 2}")
                        tiles[w] = g_t
                        if pt == len(PT) - 1 and w == 0:
                            # split off the final query so earlier queries
                            # prefetch while q12's matrix is still landing
                            cut = np_ - S
                            nc.gpsimd.dma_start(
                                g_t[:cut, :].rearrange("p (w j) -> p w j", j=T),
                                dsc_p[p0:p0 + cut,
                                      base_row:base_row + W, :])
                            nc.gpsimd.dma_start(
                                g_t[cut:np_, :].rearrange("p (w j) -> p w j", j=T),
                                dsc_p[p0 + cut:p0 + np_,
                                      base_row:base_row + W, :])
                        else:
                            nc.gpsimd.dma_start(
                                g_t[:np_, :].rearrange("p (w j) -> p w j", j=T),
                                dsc_p[p0:p0 + np_,
                                      base_row:base_row + W, :])

                st.append((p0, np_, r_a, r_b, gf_tiles, gb_tiles))

            eng = nc.vector

            def fused_mud(pt, np_, prev_t):
                # seg0: min(F[i-1,j], F[i-1,j-1]); seg1: min(B[i+1,j], B[i+1,j+1])
                m = mudp.tile([128, 2 * T], f16, tag=f"m{pt}")
                pv = prev_t[:np_, :].rearrange("p (s c) -> p s c", s=2)
                mv = m[:np_, :].rearrange("p (s c) -> p s c", s=2)
                eng.tensor_tensor(
                    mv[:, :, 0:T], pv[:, :, 1:T + 1], pv[:, :, 0:T], MIN)
                return m

            # round-robin issue across streams: the DVE runs its scheduled
            # order in-order, so interleaving lets ready streams fill the
            # latency gaps of streams that are serial-chain-bound.
            for t in range(HT):
                for pt, (p0, np_, r_a, r_b, gf_tiles, gb_tiles) in enumerate(st):
                    prev, cur = (r_a, r_b) if t % 2 == 0 else (r_b, r_a)
                    if pt == 0 or (pt == 2 and t >= 40):
                        # stream 0 runs mostly solo (other streams' data not
                        # staged yet): split the mud so the fwd/bwd chains
                        # decouple -- cadence becomes engine-bound (4 ops)
                        # instead of fused-chain latency-bound.
                        mud = mudp.tile([128, 2 * T], f16, tag=f"m{pt}")
                        eng.tensor_tensor(
                            mud[:np_, 0:T], prev[:np_, 1:T + 1],
                            prev[:np_, 0:T], MIN)
                        eng.tensor_tensor_scan(
                            cur[:np_, 1:T + 1], mud[:np_, 0:T],
                            gf_tiles[t // W][:np_, (t % W) * T:(t % W + 1) * T],
                            BIG, MIN, ADD)
                        eng.tensor_tensor(
                            mud[:np_, T:2 * T], prev[:np_, T + 1:2 * T + 1],
                            prev[:np_, T + 2:2 * T + 2], MIN)
                    else:
                        mud = fused_mud(pt, np_, prev)
                        # fwd row i=t (left-to-right)
                        eng.tensor_tensor_scan(
                            cur[:np_, 1:T + 1], mud[:np_, 0:T],
                            gf_tiles[t // W][:np_, (t % W) * T:(t % W + 1) * T],
                            BIG, MIN, ADD)
                    # bwd row i=127-t (right-to-left via reversed APs)
                    slot = (W - 1) - (t % W)
                    gb_row = gb_tiles[t // W][:np_, slot * T:(slot + 1) * T]
                    eng.tensor_tensor_scan(
                        cur[:np_, 2 * T:T:-1],
                        mud[:np_, 2 * T - 1:T - 1:-1],
                        gb_row[:, T - 1::-1], BIG, MIN, ADD)
                    if t == 0:
                        # row-0 buffers become interior rows: corners -> BIG
                        eng.memset(prev[:np_, 0:1], BIG)
                        eng.memset(prev[:np_, 2 * T + 1:2 * T + 2], BIG)

            for pt, (p0, np_, r_a, r_b, gf_tiles, gb_tiles) in enumerate(st):
                # HT=64 even: last t=63 odd -> cur=r_a holds F[63] and B[64]
                fin = r_a if HT % 2 == 0 else r_b
                mud64 = fused_mud(pt, np_, fin)  # seg1 = min(B[64,*])
                comb = mudp.tile([128, T], f32, tag=f"c{pt % 2}")
                eng.tensor_tensor(
                    comb[:np_, :], fin[:np_, 1:T + 1], mud64[:np_, T:2 * T],
                    ADD)
                cd = dpp.tile([128, 1], f32, tag=f"cd{pt}")
                eng.tensor_reduce(cd[:np_, :], comb[:np_, :],
                                  mybir.AxisListType.X, MIN)
                nc.sync.dma_start(out_flat[p0:p0 + np_], cd[:np_, 0:1])

    ents = getattr(tc, "_perfetto_entries", None)
    if ents:
        _predicted_ns = int(max(e[2] for e in ents))
    nc.compile()
    return nc


def _pack_inputs(X, Yf):
    """Host-side packing into the exact SBUF layouts the kernel DMAs 1:1."""
    e4 = ml_dtypes.float8_e4m3
    # Whole pipeline at 1/16 scale: x,y each carry 1/4 (exact in fp8),
    # norms carry 1/16 (exact in fp32). Host rescales logits by 16.
    # xt: [QPAD, 128(dk), NK*T] = fp8(-2X/4)^T, K-tile-major free dim
    Xp = np.zeros((QPAD, T, DD), np.float32)
    Xp[:Q] = X
    xtq = np.ascontiguousarray(
        (-0.5 * Xp).astype(e4).transpose(0, 2, 1)         # [QPAD, DD, T]
        .reshape(QPAD, NK, 128, T).transpose(0, 2, 1, 3)  # [QPAD, 128, NK, T]
        .reshape(QPAD, 128, NK * T))
    # yt: [128(dk), NK*SJ] = fp8(Y/4)^T
    yt = np.ascontiguousarray(
        (0.25 * Yf).astype(e4).transpose(2, 0, 1)         # [DD, S, T]
        .reshape(NK, 128, SJ).transpose(1, 0, 2)          # [128, NK, SJ]
        .reshape(128, NK * SJ))
    # exact fp32 norms (1/16-scaled)
    x2 = np.einsum("qtd,qtd->qt", Xp, Xp, dtype=np.float32) / SCALE
    y2 = np.einsum("std,std->st", Yf, Yf, dtype=np.float32) / SCALE
    augl = np.zeros((QPAD, 2, T), np.float32)
    augl[:, 0, :] = 1.0
    augl[:, 1, :] = x2
    augr = np.zeros((2, SJ), np.float32)
    augr[0] = y2.reshape(SJ)
    augr[1] = 1.0
    return xtq, yt, augl, augr


def kernel(support_features, support_labels, target_features, n_classes):
    global _built
    from concourse.bass_utils import run_bass_kernel_spmd

    X = np.asarray(target_features, dtype=np.float32)
    Yf = np.asarray(support_features, dtype=np.float32)
    labels = np.asarray(support_labels)
    ncls = int(np.asarray(n_classes))
    assert X.shape == (Q, T, DD) and Yf.shape == (S, T, DD), (
        f"kernel compiled for fixed shapes; got {X.shape}, {Yf.shape}")

    xtq, yt, augl, augr = _pack_inputs(X, Yf)

    if _built is None:
        _built = _build()
    nc = _built

    in_maps = [
        {
            "xt": np.ascontiguousarray(xtq[c * QC:(c + 1) * QC]),
            "yt": yt,
            "augl": np.ascontiguousarray(augl[c * QC:(c + 1) * QC]),
            "augr": augr,
        }
        for c in range(NCORES)
    ]
    res = run_bass_kernel_spmd(nc, in_maps, list(range(NCORES)))
    global _last_result
    _last_result = res
    cum = np.concatenate([res.results[c]["out_cd"] for c in range(NCORES)])[:Q]

    onehot = (labels[:, None] == np.arange(ncls)[None, :]).astype(np.float32)
    counts = np.maximum(onehot.sum(axis=0), 1.0).astype(np.float32)
    logits = -(cum.astype(np.float32) * SCALE @ onehot) / counts
    return logits.astype(np.float32)
